# revision 12
# baseline (speedup 1.0000x reference)
"""Trainium2 Bass kernel for AdaptiveHierarchicalAttention (8 NeuronCores).

Reference computation (per level l in 0..3):
    x_l = query[:, ::2^l, :]                         # [1, S_l, E], S_l = S >> l
    outs[l] = MHA_l(x_l)                             # 16-head self-attention
Bottom-up: current = outs[3]; for l in (2,1,0):
    current = upsample_linear(current, S_l) @ up_w[l].T + up_b[l] + outs[l]

Sharding (8 cores):
  - QKV projections + attention: tensor-parallel over heads (2 heads/core).
    Scores are computed transposed (scoresT[k, q] = K @ Q^T, feature-major
    Q/K straight out of the QKV matmul), exp on ScalarE without max
    subtraction (scores are O(1) for this problem), and A = attnT^T @ V via
    an AV matmul whose lhsT is token-major V with an appended ones column,
    which yields the softmax denominator for free.
  - Per level, normalized attention outputs (feature-major, 128 feature rows
    per core) are exchanged pre-windowed per destination core: levels 1-3
    share ONE fused AllToAll issued before level-0 attention so it overlaps
    the level-0 compute, and level 0 uses an AllToAll at the end.
  - Epilogue: the up-propagation chain is LINEAR, so it is folded on the
    host: final = sum_l U_{l->0}(A_l @ D_l) + beta, with
    D_l = W_out[l]^T @ up_w[l-1]^T @ ... @ up_w[0]^T and beta the folded
    bias chain. The device multiplies each level's gathered attention
    window by D_l at the level's own (coarse) resolution, then applies the
    iterated 2x linear-interp upsampling on the Vector engine while
    accumulating level by level. Halo columns beyond the global sequence
    edges are edge-replicated in the exchange payload, reproducing the
    reference's clipped interpolation exactly. The folded D_l weights are
    prefetched during the attention phase so the epilogue is never
    DMA-bound, and levels 3..1 of the epilogue only depend on the first
    collective, so they execute in the shadow of the level-0 collective.

kernel(**inputs) takes the FULL unsharded inputs and returns the FULL output.
"""

import sys

import numpy as np

sys.path.insert(0, "/opt/trn_rl_repo")

import ml_dtypes  # noqa: E402

import concourse.mybir as mybir  # noqa: E402
import concourse.tile as tile  # noqa: E402
from concourse import bacc  # noqa: E402
from concourse.masks import make_identity  # noqa: E402

F32 = mybir.dt.float32
BF16 = mybir.dt.bfloat16
BF16_NP = ml_dtypes.bfloat16

NCORES = 8
LEVELS = 4
P = 128


def _cfg(S=2048, E=1024, H=16):
    c = {}
    c["S"], c["E"], c["H"] = S, E, H
    c["HD"] = E // H                    # head dim
    c["HPC"] = H // NCORES              # heads per core
    c["F"] = c["HPC"] * c["HD"]         # feature rows per core
    assert c["F"] == 128, "per-core feature slice must be 128"
    c["ECH"] = E // P                   # contraction chunks
    c["SL"] = [S >> l for l in range(LEVELS)]
    c["LOFF"] = np.cumsum([0] + c["SL"]).tolist()   # level offsets in token concat
    c["T"] = sum(c["SL"])               # total tokens across levels
    c["CH"] = [sl // P for sl in c["SL"]]
    c["CHOFF"] = np.cumsum([0] + c["CH"]).tolist()
    c["CHT"] = sum(c["CH"])
    c["BLK"] = [sl // NCORES for sl in c["SL"]]     # per-core token block
    # epilogue windows (token ranges incl. halos): level 0 has no halo.
    c["WIN"] = [c["BLK"][0], c["BLK"][1] + 2, c["BLK"][2] + 4, c["BLK"][3] + 4]
    # upsample phase per step l+1 -> l  (True = "even" pattern A)
    c["PHASE_A"] = [True, False, True]  # index by l of target level 0,1,2
    c["PAD"] = 2
    c["QB0"] = min(512, c["SL"][0])     # level-0 q-block width
    return c


# ---------------------------------------------------------------------------
# builder
# ---------------------------------------------------------------------------

def build(cfg, kgroup=8):
    S, E = cfg["S"], cfg["E"]
    HD, F, ECH = cfg["HD"], cfg["F"], cfg["ECH"]
    SL, LOFF, T = cfg["SL"], cfg["LOFF"], cfg["T"]
    CH, CHOFF, CHT = cfg["CH"], cfg["CHOFF"], cfg["CHT"]
    BLK, WIN, PAD = cfg["BLK"], cfg["WIN"], cfg["PAD"]
    QB0 = cfg["QB0"]
    NCK0 = SL[0] // QB0                 # number of level-0 q-blocks
    FT = ECH  # number of 128-wide feature tiles of E
    VW = 2 * HD + 4  # V-token chunk width: [V_A | 1 | pad | V_B | 1 | pad]

    nc = bacc.Bacc(
        "TRN2",
        target_bir_lowering=False,
        debug=False,
        enable_asserts=False,
        num_devices=NCORES,
    )

    # --- I/O ---------------------------------------------------------------
    qT = nc.dram_tensor("qT", [E, S], BF16, kind="ExternalInput")
    win_p = nc.dram_tensor("win", [LEVELS, P, 3, ECH, F], BF16, kind="ExternalInput")
    bin_p = nc.dram_tensor("bin", [P, LEVELS, 3], F32, kind="ExternalInput")
    # folded epilogue weights D_l (E x E each) packed for lhsT use, + beta
    wd_p = nc.dram_tensor("wd", [LEVELS, P, ECH, FT, P], BF16, kind="ExternalInput")
    beta_p = nc.dram_tensor("beta", [P, FT], F32, kind="ExternalInput")
    out_p = nc.dram_tensor("out", [E, BLK[0]], F32, kind="ExternalOutput")

    # --- internal DRAM (collective bounce) ---------------------------------
    # levels 1..3 are exchanged pre-windowed per destination: the bounce-write
    # DMA materializes per-dest overlapping window shards and one AllToAll
    # per level delivers them (levels are exchanged as soon as their
    # attention completes so only the last, smallest exchange is exposed).
    CW = [SL[3] + 2 * PAD, SL[2] + 2 * PAD, SL[1] + 2 * PAD]
    CO = {3: 0, 2: CW[0], 1: CW[0] + CW[1]}      # concat offset per level
    CTOT = sum(CW)
    HALO = {1: 1, 2: 2, 3: 2}
    agin = {
        l: nc.dram_tensor(f"agin{l}", [NCORES, P, WIN[l]], BF16) for l in (1, 2, 3)
    }
    gx = {l: nc.dram_tensor(f"g{l}", [NCORES, P, WIN[l]], BF16) for l in (1, 2, 3)}
    # level 0: each core wants exactly its own 256-col block of every core's
    # head-slice -> AllToAll
    agin[0] = nc.dram_tensor("agin0", [NCORES, P, BLK[0]], BF16)
    gx[0] = nc.dram_tensor("g0", [NCORES, P, BLK[0]], BF16)
    rg = [list(range(NCORES))]

    with tile.TileContext(nc) as tc:
        from contextlib import ExitStack

        with ExitStack() as ctx:
            pool = lambda name, bufs, **kw: ctx.enter_context(
                tc.tile_pool(name=name, bufs=bufs, **kw)
            )
            const = pool("const", 1)
            # folded epilogue weights live for the whole kernel (prefetched
            # during attention); the epilogue work pools are created after
            # the attention pools close and reuse their SBUF/PSUM space.
            d_pool = pool("dw", 1)

            stackA = ctx.enter_context(ExitStack())
            poolA = lambda name, bufs, **kw: stackA.enter_context(
                tc.tile_pool(name=name, bufs=bufs, **kw)
            )
            qk_pool = poolA("qk", 1)
            vf_pool = poolA("vf", 1)
            at_pool = poolA("at", 8)
            nrm_pool = poolA("nrm", 2)
            a0_pool = poolA("a0", 2)
            qkv_ps = poolA("qkv_ps", 1, space="PSUM")
            tr_ps = poolA("tr_ps", 1, space="PSUM")
            sc_ps = poolA("sc_ps", 2, space="PSUM")
            av_ps = poolA("av_ps", 2, space="PSUM")

            # --- constants / persistent buffers ---------------------------
            b_sb = const.tile([P, LEVELS, 3], F32, tag="b_sb")
            nc.sync.dma_start(b_sb[:], bin_p[:])
            beta_sb = const.tile([P, FT], F32, tag="beta_sb")
            nc.sync.dma_start(beta_sb[:], beta_p[:])

            ident = const.tile([P, P], BF16, tag="ident")
            make_identity(nc, ident[:])
            ones_sb = const.tile([P, HD], BF16, tag="ones")
            nc.vector.memset(ones_sb[:], 1.0)

            # QKV weights for all levels (persistent; level-3 slice first so
            # compute can start ASAP, then the query transpose, then the rest)
            wl_sb = const.tile([P, LEVELS, 3, ECH, F], BF16, tag="wl_sb")
            win_r = win_p.ap().rearrange("l p three c f -> p l three c f")
            nc.sync.dma_start(wl_sb[:, 3], win_r[:, 3])

            xT = qk_pool.tile([P, ECH, S], BF16, tag="xT")
            qT_r = qT.ap().rearrange("(c p) t -> p c t", p=P)
            for c in range(ECH):
                nc.sync.dma_start(xT[:, c, :], qT_r[:, c, :])
            for l in (2, 1, 0):
                nc.sync.dma_start(wl_sb[:, l], win_r[:, l])

            # prefetch folded epilogue weights in per-chunk pieces (keeps
            # head-of-line blocking on the DMA queue small)
            wd_sb = d_pool.tile([P, LEVELS, ECH, FT, P], BF16, tag="wd_sb")
            wd_r = wd_p.ap().rearrange("l p c ft f -> p l c ft f")
            for l in (3, 2, 1, 0):
                for c in range(ECH):
                    nc.sync.dma_start(wd_sb[:, l, c], wd_r[:, l, c])

            Q = qk_pool.tile([P, T], BF16, tag="Q")
            K = qk_pool.tile([P, T], BF16, tag="K")
            Vt = qk_pool.tile([P, CHT, VW], BF16, tag="Vt")
            nc.vector.memset(Vt[:, :, HD : HD + 1], 1.0)
            nc.vector.memset(Vt[:, :, 2 * HD + 2 : 2 * HD + 3], 1.0)

            # ---------------- per-level QKV + attention -------------------
            def qkv_level(l):
                stride = 1 << l
                sl = SL[l]
                nt = min(512, sl)
                vfeat = vf_pool.tile([F, SL[0]], BF16, tag="vf")
                for part, dst in ((0, Q), (1, K), (2, vfeat)):
                    for n0 in range(0, sl, nt):
                        ps = qkv_ps.tile([F, nt], F32, tag="qkv")
                        for c in range(ECH):
                            rhs = xT[:, c, n0 * stride : (n0 + nt) * stride : stride]
                            nc.tensor.matmul(
                                ps[:],
                                lhsT=wl_sb[:, l, part, c, :],
                                rhs=rhs,
                                start=(c == 0),
                                stop=(c == ECH - 1),
                            )
                        if part < 2:
                            o = dst[:, LOFF[l] + n0 : LOFF[l] + n0 + nt]
                        else:
                            o = dst[:, n0 : n0 + nt]
                        nc.vector.tensor_tensor(
                            o,
                            ps[:],
                            b_sb[:, l, part : part + 1].to_broadcast((F, nt)),
                            mybir.AluOpType.add,
                        )
                # V -> token-major (PE transpose)
                for j in range(CH[l]):
                    tp = tr_ps.tile([P, F], BF16, tag="tr")
                    nc.tensor.transpose(tp[:], vfeat[:, j * P : (j + 1) * P], ident[:F, :F])
                    ch = CHOFF[l] + j
                    nc.vector.tensor_copy(out=Vt[:, ch, 0:HD], in_=tp[:, 0:HD])
                    nc.vector.tensor_copy(
                        out=Vt[:, ch, HD + 2 : 2 * HD + 2], in_=tp[:, HD : 2 * HD]
                    )

            def attn_block(l, qb0, qbw, a_dst, a_off):
                """Attention for q-block [qb0, qb0+qbw) of level l -> a_dst[:, a_off:]."""
                qsl = slice(LOFF[l] + qb0, LOFF[l] + qb0 + qbw)
                nch = CH[l]
                avA = av_ps.tile([HD + 1, qbw], F32, tag="av")
                avB = av_ps.tile([HD + 1, qbw], F32, tag="av")
                for g0_ in range(0, nch, kgroup):
                    gch = list(range(g0_, min(g0_ + kgroup, nch)))
                    ats = {}
                    # score chunks in pairs: one 2-bank PSUM tile, one exp
                    # instruction per pair (amortizes ScalarE per-op cost)
                    for i0 in range(0, len(gch), 2):
                        pair = gch[i0 : i0 + 2]
                        for h in (0, 1):
                            b = h * HD
                            sp = sc_ps.tile([P, 2 * qbw], F32, tag="sc")
                            for j, kc in enumerate(pair):
                                nc.tensor.matmul(
                                    sp[:, j * qbw : (j + 1) * qbw],
                                    lhsT=K[b : b + HD, LOFF[l] + kc * P : LOFF[l] + (kc + 1) * P],
                                    rhs=Q[b : b + HD, qsl],
                                    start=True,
                                    stop=True,
                                )
                            at = at_pool.tile([P, 2 * qbw], BF16, tag="at")
                            nc.scalar.activation(
                                at[:, 0 : len(pair) * qbw],
                                sp[:, 0 : len(pair) * qbw],
                                mybir.ActivationFunctionType.Exp,
                            )
                            for j, kc in enumerate(pair):
                                ats[(kc, h)] = at[:, j * qbw : (j + 1) * qbw]
                    for kc in gch:
                        for h, av in ((0, avA), (1, avB)):
                            c0 = 0 if h == 0 else HD + 2
                            nc.tensor.matmul(
                                av[:],
                                lhsT=Vt[:, CHOFF[l] + kc, c0 : c0 + HD + 1],
                                rhs=ats[(kc, h)],
                                start=(kc == 0),
                                stop=(kc == nch - 1),
                            )

                def _norm_bc(av):
                    dn = nrm_pool.tile([P, qbw], BF16, tag="dn")
                    nc.vector.tensor_copy(out=dn[HD : HD + 1, :], in_=av[HD : HD + 1, :])
                    with nc.allow_low_precision(
                        reason="softmax denominators tolerate bf16 recip"
                    ):
                        nc.vector.reciprocal(dn[HD : HD + 1, :], dn[HD : HD + 1, :])
                    bc_ps = tr_ps.tile([HD, qbw], F32, tag="tr")
                    nc.tensor.matmul(
                        bc_ps[:],
                        lhsT=ones_sb[HD : HD + 1, 0:HD],
                        rhs=dn[HD : HD + 1, :],
                        start=True,
                        stop=True,
                    )
                    bc = nrm_pool.tile([HD, qbw], F32, tag="bc_sb")
                    nc.vector.tensor_copy(out=bc[:], in_=bc_ps[:])
                    return bc

                bcA = _norm_bc(avA)
                nc.vector.tensor_mul(
                    out=a_dst[0:HD, a_off : a_off + qbw], in0=avA[0:HD, :], in1=bcA[:]
                )
                bcB = _norm_bc(avB)
                tmpB = nrm_pool.tile([HD, qbw], BF16, tag="tmpB")
                nc.vector.tensor_mul(out=tmpB[:], in0=avB[0:HD, :], in1=bcB[:])
                # head B rows live at partitions HD..2HD: shift via DMA
                nc.sync.dma_start(a_dst[HD : 2 * HD, a_off : a_off + qbw], tmpB[:])

            A123 = qk_pool.tile([P, CTOT], BF16, tag="A123")

            def attn_level_whole(l):
                """Levels 1..3: write into the fused concat buffer (padded)."""
                sl = SL[l]
                co = CO[l]
                qbw = min(512, sl)
                for qb0 in range(0, sl, qbw):
                    attn_block(l, qb0, qbw, A123, co + PAD + qb0)
                nc.vector.tensor_copy(
                    out=A123[:, co : co + PAD],
                    in_=A123[:, co + PAD : co + PAD + 1].to_broadcast((P, PAD)),
                )
                nc.vector.tensor_copy(
                    out=A123[:, co + PAD + sl : co + 2 * PAD + sl],
                    in_=A123[:, co + PAD + sl - 1 : co + PAD + sl].to_broadcast((P, PAD)),
                )

            def gather_level(l):
                """One bounce DMA (all 8 overlapping dest windows) + AllToAll."""
                s0 = CO[l] + PAD - HALO[l]
                src = A123[:, s0 : s0 + WIN[l]]
                # insert a dest dim with stride BLK[l] (windows overlap by the
                # halos) so a single DMA materializes all 8 dest shards
                src.ap.insert(1, [BLK[l], NCORES])
                nc.sync.dma_start(agin[l].ap().rearrange("d p w -> p d w"), src)
                nc.gpsimd.collective_compute(
                    "AllToAll",
                    mybir.AluOpType.bypass,
                    replica_groups=rg,
                    ins=[agin[l][:]],
                    outs=[gx[l][:]],
                )

            def attn_level0():
                """Level 0: no pads (no halo needed), per-block bounce writes."""
                ndst = QB0 // BLK[0]
                for b in range(NCK0):
                    A0 = a0_pool.tile([P, QB0], BF16, tag="A0")
                    attn_block(0, b * QB0, QB0, A0, 0)
                    # bounce-write this block's columns to their dest slots
                    nc.sync.dma_start(
                        agin[0].ap()[b * ndst : (b + 1) * ndst].rearrange(
                            "d p w -> p d w"
                        ),
                        A0[:].rearrange("p (d w) -> p d w", d=ndst),
                    )
                nc.gpsimd.collective_compute(
                    "AllToAll",
                    mybir.AluOpType.bypass,
                    replica_groups=rg,
                    ins=[agin[0][:]],
                    outs=[gx[0][:]],
                )

            # ---------------- epilogue ------------------------------------
            # Z_l = (gathered A_l window) @ D_l at level-l resolution, then
            # chained 2x upsample + accumulate on DVE:
            #   acc_3 = Z_3; acc_l = U(acc_{l+1}) + Z_l; out = acc_0 + beta
            def z_level(gtile, goff, l, w):
                """Matmul Z_l -> list of psum tiles (one per ft)."""
                zt = acc_pool.tile([P, FT, w], BF16, tag=f"z{l}")
                for ft in range(FT):
                    ps = ep_ps.tile([P, w], F32, tag="ep")
                    for c in range(ECH):
                        nc.tensor.matmul(
                            ps[:],
                            lhsT=wd_sb[:, l, c, ft],
                            rhs=gtile[:, c, goff : goff + w],
                            start=(c == 0),
                            stop=(c == ECH - 1),
                        )
                    nc.vector.tensor_copy(out=zt[:, ft, :], in_=ps[:])
                return zt

            def z_level_add(gtile, goff, l, w, up, extra):
                """Z_l matmuls, then out_tile = psum + up (+ extra bias)."""
                res = acc_pool.tile([P, FT, w], F32 if l == 0 else BF16, tag=f"acc{l}")
                for ft in range(FT):
                    ps = ep_ps.tile([P, w], F32, tag="ep")
                    for c in range(ECH):
                        nc.tensor.matmul(
                            ps[:],
                            lhsT=wd_sb[:, l, c, ft],
                            rhs=gtile[:, c, goff : goff + w],
                            start=(c == 0),
                            stop=(c == ECH - 1),
                        )
                    nc.vector.tensor_tensor(
                        res[:, ft, :], ps[:], up[:, ft, :], mybir.AluOpType.add
                    )
                    if extra is not None:
                        nc.vector.tensor_tensor(
                            res[:, ft, :],
                            res[:, ft, :],
                            extra[:, ft : ft + 1].to_broadcast((P, w)),
                            mybir.AluOpType.add,
                        )
                return res

            def upsample(cur, ws, w, phase_a, tag):
                """2x linear-interp upsample [P, FT, ws] -> [P, FT, w] (DVE)."""
                p25 = acc_pool.tile([P, FT, ws], BF16, tag=f"p25{tag}")
                p75 = acc_pool.tile([P, FT, ws], BF16, tag=f"p75{tag}")
                nc.vector.tensor_scalar_mul(p25[:], cur[:], 0.25)
                nc.vector.tensor_scalar_mul(p75[:], cur[:], 0.75)
                up = acc_pool.tile([P, FT, w], BF16, tag=f"up{tag}")
                hw = (w + 1) // 2
                hw2 = w // 2
                if phase_a:
                    nc.vector.tensor_add(
                        up[:, :, 0::2], p25[:, :, 0:hw], p75[:, :, 1 : hw + 1]
                    )
                    nc.vector.tensor_add(
                        up[:, :, 1::2], p75[:, :, 1 : hw2 + 1], p25[:, :, 2 : hw2 + 2]
                    )
                else:
                    nc.vector.tensor_add(
                        up[:, :, 0::2], p75[:, :, 1 : hw + 1], p25[:, :, 2 : hw + 2]
                    )
                    nc.vector.tensor_add(
                        up[:, :, 1::2], p25[:, :, 1 : hw2 + 1], p75[:, :, 2 : hw2 + 2]
                    )
                return up

            # ---------------- schedule ------------------------------------
            # attention order 0, 3, 2, 1: the big level-0 exchange is issued
            # first and hides under the remaining compute; the last exchange
            # (level 1) is the only exposed one, and the epilogue chain needs
            # level 1 third, so levels 3/2 of the chain overlap its latency.
            qkv_level(0)
            attn_level0()
            qkv_level(3)
            attn_level_whole(3)
            gather_level(3)
            qkv_level(2)
            attn_level_whole(2)
            gather_level(2)
            qkv_level(1)
            attn_level_whole(1)
            gather_level(1)

            stackA.close()
            g_pool = ctx.enter_context(tc.tile_pool(name="gpool", bufs=1))
            acc_pool = ctx.enter_context(tc.tile_pool(name="acc", bufs=1))
            ep_ps = ctx.enter_context(tc.tile_pool(name="ep_ps", bufs=2, space="PSUM"))

            Gs = {}
            for l in (3, 2, 0, 1):
                w = WIN[l]
                Gs[l] = g_pool.tile([P, ECH, w], BF16, tag=f"gs{l}", name=f"gs{l}")
                nc.sync.dma_start(Gs[l][:], gx[l].ap().rearrange("b p t -> p b t"))

            # levels 3..2 run in the shadow of the level-1 collective
            acc = z_level(Gs[3], 0, 3, WIN[3])
            up = upsample(acc, WIN[3], WIN[2], cfg["PHASE_A"][2], "a")
            acc = z_level_add(Gs[2], 0, 2, WIN[2], up, None)
            up = upsample(acc, WIN[2], WIN[1], cfg["PHASE_A"][1], "b")
            acc = z_level_add(Gs[1], 0, 1, WIN[1], up, None)
            up = upsample(acc, WIN[1], WIN[0], cfg["PHASE_A"][0], "c")

            # level 0 term: per-ft matmul + add + streamed output DMA
            out_r = out_p.ap().rearrange("(c p) t -> p c t", p=P)
            w = WIN[0]
            for ft in range(FT):
                ps = ep_ps.tile([P, w], F32, tag="ep")
                for c in range(ECH):
                    nc.tensor.matmul(
                        ps[:],
                        lhsT=wd_sb[:, 0, c, ft],
                        rhs=Gs[0][:, c, :],
                        start=(c == 0),
                        stop=(c == ECH - 1),
                    )
                o = acc_pool.tile([P, w], F32, tag=f"o{ft}")
                nc.vector.tensor_tensor(o[:], ps[:], up[:, ft, :], mybir.AluOpType.add)
                nc.vector.tensor_tensor(
                    o[:],
                    o[:],
                    beta_sb[:, ft : ft + 1].to_broadcast((P, w)),
                    mybir.AluOpType.add,
                )
                nc.sync.dma_start(out_r[:, ft], o[:])

    nc.compile()
    return nc


# ---------------------------------------------------------------------------
# host-side input preparation / sharding
# ---------------------------------------------------------------------------

def make_in_maps(cfg, query, in_proj_w, in_proj_b, out_w, out_b, up_w, up_b):
    S, E, HD, F, ECH = cfg["S"], cfg["E"], cfg["HD"], cfg["F"], cfg["ECH"]
    FT = ECH
    f32 = np.float32
    f64 = np.float64

    query = np.asarray(query, f32)
    in_proj_w = np.asarray(in_proj_w, f32)
    in_proj_b = np.asarray(in_proj_b, f32)
    out_w = np.asarray(out_w, f32)
    out_b = np.asarray(out_b, f32)
    up_w = np.asarray(up_w, f32)
    up_b = np.asarray(up_b, f32)

    qT = np.ascontiguousarray(query[0].T.astype(BF16_NP))  # [E, S]

    # folded epilogue: D_l = W_out[l]^T @ up_w[l-1]^T @ ... @ up_w[0]^T
    # beta: beta_3 = b3; beta_l = beta_{l+1} @ up_w[l]^T + up_b[l] + b_l
    D = []
    for l in range(LEVELS):
        M = out_w[l].T.astype(f64)
        for j in range(l - 1, -1, -1):
            M = M @ up_w[j].T.astype(f64)
        D.append(M.astype(f32))
    Dm = np.stack(D, axis=0)  # [L, E(in), E(out)] -- already W^T layout
    beta = out_b[3].astype(f64)
    for l in range(LEVELS - 2, -1, -1):
        beta = beta @ up_w[l].T.astype(f64) + up_b[l] + out_b[l]
    beta = beta.astype(f32)

    # pack [L, e_in, e_out] -> [L, e_in%128, e_in//128, e_out//128, e_out%128]
    t = Dm.reshape(LEVELS, ECH, P, FT, P)          # [L, ec, ep, ft, fp]
    t = t.transpose(0, 2, 1, 3, 4)                 # [L, ep, ec, ft, fp]
    wd = np.ascontiguousarray(t.astype(BF16_NP))
    beta_pk = np.ascontiguousarray(beta.reshape(FT, P).T.astype(f32))  # [P, FT]

    scale = 1.0 / np.sqrt(HD).astype(f32)
    in_maps = []
    for c in range(NCORES):
        r0 = c * F
        sl_q = in_proj_w[:, r0 : r0 + F, :] * scale          # [L, F, E]
        sl_k = in_proj_w[:, E + r0 : E + r0 + F, :]
        sl_v = in_proj_w[:, 2 * E + r0 : 2 * E + r0 + F, :]
        w3 = np.stack([sl_q, sl_k, sl_v], axis=1)            # [L, 3, F, E]
        w3 = w3.transpose(0, 3, 1, 2)                        # [L, E(e), 3, F]
        w3 = w3.reshape(LEVELS, ECH, P, 3, F).transpose(0, 2, 3, 1, 4)
        w3 = np.ascontiguousarray(w3.astype(BF16_NP))        # [L, p, 3, ch, F]

        b_q = in_proj_b[:, r0 : r0 + F] * scale
        b_k = in_proj_b[:, E + r0 : E + r0 + F]
        b_v = in_proj_b[:, 2 * E + r0 : 2 * E + r0 + F]
        b3 = np.stack([b_q, b_k, b_v], axis=1)               # [L, 3, F]
        b3 = np.zeros((P, LEVELS, 3), f32) + b3.transpose(2, 0, 1)

        in_maps.append(
            {
                "qT": qT,
                "win": w3,
                "bin": np.ascontiguousarray(b3),
                "wd": wd,
                "beta": beta_pk,
            }
        )
    return in_maps


def assemble_output(cfg, results):
    S, E = cfg["S"], cfg["E"]
    blk = cfg["BLK"][0]
    out = np.empty((1, S, E), np.float32)
    for c in range(NCORES):
        out[0, c * blk : (c + 1) * blk, :] = results[c]["out"].T
    return out


_CACHE = {}


def _get_nc(cfg_key=(2048, 1024, 16)):
    if cfg_key not in _CACHE:
        cfg = _cfg(*cfg_key)
        _CACHE[cfg_key] = (cfg, build(cfg))
    return _CACHE[cfg_key]


def kernel(query, in_proj_w, in_proj_b, out_w, out_b, up_w, up_b):
    from concourse.bass_utils import run_bass_kernel_spmd

    cfg, nc = _get_nc()
    in_maps = make_in_maps(cfg, query, in_proj_w, in_proj_b, out_w, out_b, up_w, up_b)
    res = run_bass_kernel_spmd(nc, in_maps, core_ids=list(range(NCORES)))
    return assemble_output(cfg, res.results)


# revision 20
# speedup vs baseline: 1.1320x; 1.1320x over previous
"""Trainium2 Bass kernel for AdaptiveHierarchicalAttention (8 NeuronCores).

Reference computation (per level l in 0..3):
    x_l = query[:, ::2^l, :]                         # [1, S_l, E], S_l = S >> l
    outs[l] = MHA_l(x_l)                             # 16-head self-attention
Bottom-up: current = outs[3]; for l in (2,1,0):
    current = upsample_linear(current, S_l) @ up_w[l].T + up_b[l] + outs[l]

Sharding (8 cores):
  - QKV projections + attention: tensor-parallel over heads (2 heads/core).
    Scores are computed transposed (scoresT[k, q] = K @ Q^T, feature-major
    Q/K straight out of the QKV matmul), exp on ScalarE without max
    subtraction (scores are O(1) for this problem), and A = attnT^T @ V via
    an AV matmul whose lhsT is token-major V with an appended ones column,
    which yields the softmax denominator for free.
  - Per level, normalized attention outputs (feature-major, 128 feature rows
    per core) are exchanged pre-windowed per destination core: levels 1-3
    share ONE fused AllToAll issued before level-0 attention so it overlaps
    the level-0 compute, and level 0 uses an AllToAll at the end.
  - Epilogue: the up-propagation chain is LINEAR, so it is folded on the
    host: final = sum_l U_{l->0}(A_l @ D_l) + beta, with
    D_l = W_out[l]^T @ up_w[l-1]^T @ ... @ up_w[0]^T and beta the folded
    bias chain. The device multiplies each level's gathered attention
    window by D_l at the level's own (coarse) resolution, then applies the
    iterated 2x linear-interp upsampling on the Vector engine while
    accumulating level by level. Halo columns beyond the global sequence
    edges are edge-replicated in the exchange payload, reproducing the
    reference's clipped interpolation exactly. The folded D_l weights are
    prefetched during the attention phase so the epilogue is never
    DMA-bound, and levels 3..1 of the epilogue only depend on the first
    collective, so they execute in the shadow of the level-0 collective.

kernel(**inputs) takes the FULL unsharded inputs and returns the FULL output.
"""

import sys

import numpy as np

sys.path.insert(0, "/opt/trn_rl_repo")

import ml_dtypes  # noqa: E402

import concourse.mybir as mybir  # noqa: E402
import concourse.tile as tile  # noqa: E402
from concourse import bacc  # noqa: E402
from concourse.masks import make_identity  # noqa: E402

F32 = mybir.dt.float32
BF16 = mybir.dt.bfloat16
BF16_NP = ml_dtypes.bfloat16

NCORES = 8
LEVELS = 4
P = 128


def _cfg(S=2048, E=1024, H=16):
    c = {}
    c["S"], c["E"], c["H"] = S, E, H
    c["HD"] = E // H                    # head dim
    c["HPC"] = H // NCORES              # heads per core
    c["F"] = c["HPC"] * c["HD"]         # feature rows per core
    assert c["F"] == 128, "per-core feature slice must be 128"
    c["ECH"] = E // P                   # contraction chunks
    c["SL"] = [S >> l for l in range(LEVELS)]
    c["LOFF"] = np.cumsum([0] + c["SL"]).tolist()   # level offsets in token concat
    c["T"] = sum(c["SL"])               # total tokens across levels
    c["CH"] = [sl // P for sl in c["SL"]]
    c["CHOFF"] = np.cumsum([0] + c["CH"]).tolist()
    c["CHT"] = sum(c["CH"])
    c["BLK"] = [sl // NCORES for sl in c["SL"]]     # per-core token block
    # epilogue windows (token ranges incl. halos): level 0 has no halo.
    c["WIN"] = [c["BLK"][0], c["BLK"][1] + 2, c["BLK"][2] + 4, c["BLK"][3] + 4]
    # upsample phase per step l+1 -> l  (True = "even" pattern A)
    c["PHASE_A"] = [True, False, True]  # index by l of target level 0,1,2
    c["PAD"] = 2
    c["QB0"] = min(512, c["SL"][0])     # level-0 q-block width
    return c


# ---------------------------------------------------------------------------
# builder
# ---------------------------------------------------------------------------

def build(cfg, kgroup=8):
    S, E = cfg["S"], cfg["E"]
    HD, F, ECH = cfg["HD"], cfg["F"], cfg["ECH"]
    SL, LOFF, T = cfg["SL"], cfg["LOFF"], cfg["T"]
    CH, CHOFF, CHT = cfg["CH"], cfg["CHOFF"], cfg["CHT"]
    BLK, WIN, PAD = cfg["BLK"], cfg["WIN"], cfg["PAD"]
    QB0 = cfg["QB0"]
    NCK0 = SL[0] // QB0                 # number of level-0 q-blocks
    FT = ECH  # number of 128-wide feature tiles of E
    VW = 2 * HD + 4  # V-token chunk width: [V_A | 1 | pad | V_B | 1 | pad]

    nc = bacc.Bacc(
        "TRN2",
        target_bir_lowering=False,
        debug=False,
        enable_asserts=False,
        num_devices=NCORES,
    )

    # --- I/O ---------------------------------------------------------------
    qT = nc.dram_tensor("qT", [E, S], BF16, kind="ExternalInput")
    win_p = nc.dram_tensor("win", [LEVELS, P, 3, ECH, F], BF16, kind="ExternalInput")
    bin_p = nc.dram_tensor("bin", [P, LEVELS, 3], F32, kind="ExternalInput")
    # folded epilogue weights D_l (E x E each) packed for lhsT use, + beta
    wd_p = nc.dram_tensor("wd", [LEVELS, P, ECH, FT, P], BF16, kind="ExternalInput")
    beta_p = nc.dram_tensor("beta", [P, FT], F32, kind="ExternalInput")
    out_p = nc.dram_tensor("out", [E, BLK[0]], F32, kind="ExternalOutput")

    # --- internal DRAM (collective bounce) ---------------------------------
    # levels 1..3 are exchanged pre-windowed per destination: the bounce-write
    # DMA materializes per-dest overlapping window shards and one AllToAll
    # per level delivers them (levels are exchanged as soon as their
    # attention completes so only the last, smallest exchange is exposed).
    CW = [SL[3] + 2 * PAD, SL[2] + 2 * PAD, SL[1] + 2 * PAD]
    CO = {3: 0, 2: CW[0], 1: CW[0] + CW[1]}      # concat offset per level
    CTOT = sum(CW)
    HALO = {1: 1, 2: 2, 3: 2}
    # levels 3+2 share one fused exchange; level 1 gets its own (it finishes
    # last); level 0 is exchanged right after its attention so it hides under
    # level-1 compute.
    W32 = WIN[3] + WIN[2]
    agin32 = nc.dram_tensor("agin32", [NCORES, P, W32], BF16)
    g32 = nc.dram_tensor("g32", [NCORES, P, W32], BF16)
    agin1 = nc.dram_tensor("agin1", [NCORES, P, WIN[1]], BF16)
    g1 = nc.dram_tensor("g1", [NCORES, P, WIN[1]], BF16)
    agin0 = nc.dram_tensor("agin0", [NCORES, P, BLK[0]], BF16)
    g0 = nc.dram_tensor("g0", [NCORES, P, BLK[0]], BF16)
    rg = [list(range(NCORES))]

    with tile.TileContext(nc) as tc:
        from contextlib import ExitStack

        with ExitStack() as ctx:
            pool = lambda name, bufs, **kw: ctx.enter_context(
                tc.tile_pool(name=name, bufs=bufs, **kw)
            )
            const = pool("const", 1)
            # folded epilogue weights live for the whole kernel (prefetched
            # during attention); the epilogue work pools are created after
            # the attention pools close and reuse their SBUF/PSUM space.
            d_pool = pool("dw", 1)

            stackA = ctx.enter_context(ExitStack())
            poolA = lambda name, bufs, **kw: stackA.enter_context(
                tc.tile_pool(name=name, bufs=bufs, **kw)
            )
            qk_pool = poolA("qk", 1)
            vf_pool = poolA("vf", 1)
            at_pool = poolA("at", 8)
            nrm_pool = poolA("nrm", 2)
            a0_pool = poolA("a0", 2)
            qkv_ps = poolA("qkv_ps", 1, space="PSUM")
            tr_ps = poolA("tr_ps", 1, space="PSUM")
            sc_ps = poolA("sc_ps", 2, space="PSUM")
            av_ps = poolA("av_ps", 2, space="PSUM")

            # --- constants / persistent buffers ---------------------------
            b_sb = const.tile([P, LEVELS, 3], F32, tag="b_sb")
            nc.sync.dma_start(b_sb[:], bin_p[:])
            beta_sb = const.tile([P, FT], F32, tag="beta_sb")
            nc.sync.dma_start(beta_sb[:], beta_p[:])

            ident = const.tile([P, P], BF16, tag="ident")
            make_identity(nc, ident[:])
            ones_sb = const.tile([P, HD], BF16, tag="ones")
            nc.vector.memset(ones_sb[:], 1.0)

            # QKV weights for all levels (persistent; level-3 slice first so
            # compute can start ASAP, then the query transpose, then the rest)
            wl_sb = const.tile([P, LEVELS, 3, ECH, F], BF16, tag="wl_sb")
            win_r = win_p.ap().rearrange("l p three c f -> p l three c f")
            nc.sync.dma_start(wl_sb[:, 3], win_r[:, 3])

            xT = qk_pool.tile([P, ECH, S], BF16, tag="xT")
            qT_r = qT.ap().rearrange("(c p) t -> p c t", p=P)
            for c in range(ECH):
                nc.sync.dma_start(xT[:, c, :], qT_r[:, c, :])
            for l in (2, 0, 1):
                nc.sync.dma_start(wl_sb[:, l], win_r[:, l])

            # prefetch folded epilogue weights in per-chunk pieces (keeps
            # head-of-line blocking on the DMA queue small)
            wd_sb = d_pool.tile([P, LEVELS, ECH, FT, P], BF16, tag="wd_sb")
            wd_r = wd_p.ap().rearrange("l p c ft f -> p l c ft f")
            for l in (3, 2, 1, 0):
                for c in range(ECH):
                    nc.sync.dma_start(wd_sb[:, l, c], wd_r[:, l, c])

            Q = qk_pool.tile([P, T], BF16, tag="Q")
            K = qk_pool.tile([P, T], BF16, tag="K")
            Vt = qk_pool.tile([P, CHT, VW], BF16, tag="Vt")
            nc.vector.memset(Vt[:, :, HD : HD + 1], 1.0)
            nc.vector.memset(Vt[:, :, 2 * HD + 2 : 2 * HD + 3], 1.0)

            # ---------------- per-level QKV + attention -------------------
            # QKV work is emitted as a queue of small closures so attention
            # blocks can drain it into their PE bubbles (PE waits on ScalarE
            # exps between the score and AV matmuls of a block).
            def qkv_chunks(l):
                stride = 1 << l
                sl = SL[l]
                nt = min(512, sl)
                vfeat = vf_pool.tile([F, SL[0]], BF16, tag="vf", name=f"vf{l}")

                def proj(part, n0, dst):
                    def emit():
                        ps = qkv_ps.tile([F, nt], F32, tag="qkv", name="qkvps")
                        for c in range(ECH):
                            rhs = xT[:, c, n0 * stride : (n0 + nt) * stride : stride]
                            nc.tensor.matmul(
                                ps[:],
                                lhsT=wl_sb[:, l, part, c, :],
                                rhs=rhs,
                                start=(c == 0),
                                stop=(c == ECH - 1),
                            )
                        if part < 2:
                            o = dst[:, LOFF[l] + n0 : LOFF[l] + n0 + nt]
                        else:
                            o = dst[:, n0 : n0 + nt]
                        nc.vector.tensor_tensor(
                            o,
                            ps[:],
                            b_sb[:, l, part : part + 1].to_broadcast((F, nt)),
                            mybir.AluOpType.add,
                        )

                    return emit

                def vtrans(j):
                    def emit():
                        tp = tr_ps.tile([P, F], BF16, tag="tr", name="trps")
                        nc.tensor.transpose(
                            tp[:], vfeat[:, j * P : (j + 1) * P], ident[:F, :F]
                        )
                        ch = CHOFF[l] + j
                        nc.vector.tensor_copy(out=Vt[:, ch, 0:HD], in_=tp[:, 0:HD])
                        nc.vector.tensor_copy(
                            out=Vt[:, ch, HD + 2 : 2 * HD + 2], in_=tp[:, HD : 2 * HD]
                        )

                    return emit

                work = []
                for part, dst in ((0, Q), (1, K), (2, vfeat)):
                    for n0 in range(0, sl, nt):
                        work.append(proj(part, n0, dst))
                for j in range(CH[l]):
                    work.append(vtrans(j))
                return work

            def drain(work, n=None):
                k = len(work) if n is None else min(n, len(work))
                for _ in range(k):
                    work.pop(0)()

            def attn_block(l, qb0, qbw, a_dst, a_off, filler=None):
                """Attention for q-block [qb0, qb0+qbw) of level l -> a_dst[:, a_off:]."""
                qsl = slice(LOFF[l] + qb0, LOFF[l] + qb0 + qbw)
                nch = CH[l]
                avA = av_ps.tile([HD + 1, qbw], F32, tag="av")
                avB = av_ps.tile([HD + 1, qbw], F32, tag="av")
                for g0_ in range(0, nch, kgroup):
                    gch = list(range(g0_, min(g0_ + kgroup, nch)))
                    ats = {}
                    # score chunks in pairs: one 2-bank PSUM tile, one exp
                    # instruction per pair (amortizes ScalarE per-op cost)
                    for i0 in range(0, len(gch), 2):
                        pair = gch[i0 : i0 + 2]
                        for h in (0, 1):
                            b = h * HD
                            sp = sc_ps.tile([P, 2 * qbw], F32, tag="sc")
                            for j, kc in enumerate(pair):
                                nc.tensor.matmul(
                                    sp[:, j * qbw : (j + 1) * qbw],
                                    lhsT=K[b : b + HD, LOFF[l] + kc * P : LOFF[l] + (kc + 1) * P],
                                    rhs=Q[b : b + HD, qsl],
                                    start=True,
                                    stop=True,
                                )
                            at = at_pool.tile([P, 2 * qbw], BF16, tag="at")
                            nc.scalar.activation(
                                at[:, 0 : len(pair) * qbw],
                                sp[:, 0 : len(pair) * qbw],
                                mybir.ActivationFunctionType.Exp,
                            )
                            for j, kc in enumerate(pair):
                                ats[(kc, h)] = at[:, j * qbw : (j + 1) * qbw]
                    if filler is not None:
                        filler()
                    for kc in gch:
                        for h, av in ((0, avA), (1, avB)):
                            c0 = 0 if h == 0 else HD + 2
                            nc.tensor.matmul(
                                av[:],
                                lhsT=Vt[:, CHOFF[l] + kc, c0 : c0 + HD + 1],
                                rhs=ats[(kc, h)],
                                start=(kc == 0),
                                stop=(kc == nch - 1),
                            )

                def _norm_bc(av):
                    dn = nrm_pool.tile([P, qbw], BF16, tag="dn")
                    nc.vector.tensor_copy(out=dn[HD : HD + 1, :], in_=av[HD : HD + 1, :])
                    with nc.allow_low_precision(
                        reason="softmax denominators tolerate bf16 recip"
                    ):
                        nc.vector.reciprocal(dn[HD : HD + 1, :], dn[HD : HD + 1, :])
                    bc_ps = tr_ps.tile([HD, qbw], F32, tag="tr")
                    nc.tensor.matmul(
                        bc_ps[:],
                        lhsT=ones_sb[HD : HD + 1, 0:HD],
                        rhs=dn[HD : HD + 1, :],
                        start=True,
                        stop=True,
                    )
                    bc = nrm_pool.tile([HD, qbw], F32, tag="bc_sb")
                    nc.vector.tensor_copy(out=bc[:], in_=bc_ps[:])
                    return bc

                bcA = _norm_bc(avA)
                nc.vector.tensor_mul(
                    out=a_dst[0:HD, a_off : a_off + qbw], in0=avA[0:HD, :], in1=bcA[:]
                )
                bcB = _norm_bc(avB)
                tmpB = nrm_pool.tile([HD, qbw], BF16, tag="tmpB")
                nc.vector.tensor_mul(out=tmpB[:], in0=avB[0:HD, :], in1=bcB[:])
                # head B rows live at partitions HD..2HD: shift via DMA
                nc.sync.dma_start(a_dst[HD : 2 * HD, a_off : a_off + qbw], tmpB[:])

            A123 = qk_pool.tile([P, CTOT], BF16, tag="A123")

            def attn_level_whole(l, work):
                """Levels 1..3: write into the fused concat buffer (padded)."""
                sl = SL[l]
                co = CO[l]
                qbw = min(512, sl)
                nblk = sl // qbw
                groups = nblk * -(-CH[l] // kgroup)
                per = min(4, -(-len(work) // groups)) if work else 0
                for qb0 in range(0, sl, qbw):
                    attn_block(
                        l, qb0, qbw, A123, co + PAD + qb0,
                        filler=lambda: drain(work, per),
                    )
                nc.vector.tensor_copy(
                    out=A123[:, co : co + PAD],
                    in_=A123[:, co + PAD : co + PAD + 1].to_broadcast((P, PAD)),
                )
                nc.vector.tensor_copy(
                    out=A123[:, co + PAD + sl : co + 2 * PAD + sl],
                    in_=A123[:, co + PAD + sl - 1 : co + PAD + sl].to_broadcast((P, PAD)),
                )

            def bounce_windows(l, dst_dram, woff):
                """One DMA materializing all 8 overlapping dest windows."""
                s0 = CO[l] + PAD - HALO[l]
                src = A123[:, s0 : s0 + WIN[l]]
                src.ap.insert(1, [BLK[l], NCORES])
                dst = dst_dram.ap().rearrange("d p w -> p d w")
                nc.sync.dma_start(dst[:, :, woff : woff + WIN[l]], src)

            def a2a(ins_t, outs_t):
                nc.gpsimd.collective_compute(
                    "AllToAll",
                    mybir.AluOpType.bypass,
                    replica_groups=rg,
                    ins=[ins_t[:]],
                    outs=[outs_t[:]],
                )

            def attn_level0(work):
                """Level 0: no pads (no halo needed), per-block bounce writes."""
                ndst = QB0 // BLK[0]
                per = (len(work) + 2 * NCK0 - 1) // (2 * NCK0) if work else 0
                for b in range(NCK0):
                    A0 = a0_pool.tile([P, QB0], BF16, tag="A0")
                    attn_block(
                        0, b * QB0, QB0, A0, 0, filler=lambda: drain(work, per)
                    )
                    # bounce-write this block's columns to their dest slots
                    nc.sync.dma_start(
                        agin0.ap()[b * ndst : (b + 1) * ndst].rearrange(
                            "d p w -> p d w"
                        ),
                        A0[:].rearrange("p (d w) -> p d w", d=ndst),
                    )
                a2a(agin0, g0)

            # ---------------- epilogue ------------------------------------
            # Z_l = (gathered A_l window) @ D_l at level-l resolution, then
            # chained 2x upsample + accumulate on DVE:
            #   acc_3 = Z_3; acc_l = U(acc_{l+1}) + Z_l; out = acc_0 + beta
            def z_level(gtile, goff, l, w):
                """Matmul Z_l -> list of psum tiles (one per ft)."""
                zt = acc_pool.tile([P, FT, w], BF16, tag=f"z{l}")
                for ft in range(FT):
                    ps = ep_ps.tile([P, w], F32, tag="ep")
                    for c in range(ECH):
                        nc.tensor.matmul(
                            ps[:],
                            lhsT=wd_sb[:, l, c, ft],
                            rhs=gtile[:, c, goff : goff + w],
                            start=(c == 0),
                            stop=(c == ECH - 1),
                        )
                    nc.vector.tensor_copy(out=zt[:, ft, :], in_=ps[:])
                return zt

            def z_level_add(gtile, goff, l, w, up, extra):
                """Z_l matmuls, then out_tile = psum + up (+ extra bias)."""
                res = acc_pool.tile([P, FT, w], F32 if l == 0 else BF16, tag=f"acc{l}")
                for ft in range(FT):
                    ps = ep_ps.tile([P, w], F32, tag="ep")
                    for c in range(ECH):
                        nc.tensor.matmul(
                            ps[:],
                            lhsT=wd_sb[:, l, c, ft],
                            rhs=gtile[:, c, goff : goff + w],
                            start=(c == 0),
                            stop=(c == ECH - 1),
                        )
                    nc.vector.tensor_tensor(
                        res[:, ft, :], ps[:], up[:, ft, :], mybir.AluOpType.add
                    )
                    if extra is not None:
                        nc.vector.tensor_tensor(
                            res[:, ft, :],
                            res[:, ft, :],
                            extra[:, ft : ft + 1].to_broadcast((P, w)),
                            mybir.AluOpType.add,
                        )
                return res

            def upsample(cur, ws, w, phase_a, tag):
                """2x linear-interp upsample [P, FT, ws] -> [P, FT, w] (DVE)."""
                p25 = acc_pool.tile([P, FT, ws], BF16, tag=f"p25{tag}")
                p75 = acc_pool.tile([P, FT, ws], BF16, tag=f"p75{tag}")
                nc.vector.tensor_scalar_mul(p25[:], cur[:], 0.25)
                nc.vector.tensor_scalar_mul(p75[:], cur[:], 0.75)
                up = acc_pool.tile([P, FT, w], BF16, tag=f"up{tag}")
                hw = (w + 1) // 2
                hw2 = w // 2
                if phase_a:
                    nc.vector.tensor_add(
                        up[:, :, 0::2], p25[:, :, 0:hw], p75[:, :, 1 : hw + 1]
                    )
                    nc.vector.tensor_add(
                        up[:, :, 1::2], p75[:, :, 1 : hw2 + 1], p25[:, :, 2 : hw2 + 2]
                    )
                else:
                    nc.vector.tensor_add(
                        up[:, :, 0::2], p75[:, :, 1 : hw + 1], p25[:, :, 2 : hw + 2]
                    )
                    nc.vector.tensor_add(
                        up[:, :, 1::2], p25[:, :, 1 : hw2 + 1], p75[:, :, 2 : hw2 + 2]
                    )
                return up

            # ---------------- schedule ------------------------------------
            # level order 3, 2, 0, 1: the fused 3+2 exchange goes out early;
            # the big level-0 exchange hides under level-1 compute; only the
            # small level-1 exchange is exposed, and the epilogue is ordered
            # so just Z_1 + one upsample + final add remain after it lands.
            drain(qkv_chunks(3))
            w2 = qkv_chunks(2)
            attn_level_whole(3, w2)
            drain(w2)
            w0 = qkv_chunks(0)
            attn_level_whole(2, w0)
            bounce_windows(3, agin32, 0)
            bounce_windows(2, agin32, WIN[3])
            a2a(agin32, g32)
            drain(w0)
            w1 = qkv_chunks(1)
            attn_level0(w1)
            drain(w1)
            attn_level_whole(1, [])
            bounce_windows(1, agin1, 0)
            a2a(agin1, g1)

            stackA.close()
            g_pool = ctx.enter_context(tc.tile_pool(name="gpool", bufs=1))
            acc_pool = ctx.enter_context(tc.tile_pool(name="acc", bufs=1))
            ep_ps = ctx.enter_context(tc.tile_pool(name="ep_ps", bufs=2, space="PSUM"))

            Gs32 = g_pool.tile([P, ECH, W32], BF16, tag="gs32")
            nc.sync.dma_start(Gs32[:], g32.ap().rearrange("b p t -> p b t"))
            Gs0 = g_pool.tile([P, ECH, BLK[0]], BF16, tag="gs0")
            nc.sync.dma_start(Gs0[:], g0.ap().rearrange("b p t -> p b t"))
            Gs1 = g_pool.tile([P, ECH, WIN[1]], BF16, tag="gs1")
            nc.sync.dma_start(Gs1[:], g1.ap().rearrange("b p t -> p b t"))

            # chain part that doesn't need g1: Z3 -> up -> +Z2 -> up
            acc = z_level(Gs32, 0, 3, WIN[3])
            up = upsample(acc, WIN[3], WIN[2], cfg["PHASE_A"][2], "a")
            acc = z_level_add(Gs32, WIN[3], 2, WIN[2], up, None)
            upb = upsample(acc, WIN[2], WIN[1], cfg["PHASE_A"][1], "b")

            # level-0 term + beta (independent of g1) -> o tiles
            w = WIN[0]
            o = acc_pool.tile([P, FT, w], F32, tag="o")
            for ft in range(FT):
                ps = ep_ps.tile([P, w], F32, tag="ep")
                for c in range(ECH):
                    nc.tensor.matmul(
                        ps[:],
                        lhsT=wd_sb[:, 0, c, ft],
                        rhs=Gs0[:, c, :],
                        start=(c == 0),
                        stop=(c == ECH - 1),
                    )
                nc.vector.tensor_tensor(
                    o[:, ft, :],
                    ps[:],
                    beta_sb[:, ft : ft + 1].to_broadcast((P, w)),
                    mybir.AluOpType.add,
                )

            # tail: Z1 (+upb), upsample to 256, add into o, stream out
            acc = z_level_add(Gs1, 0, 1, WIN[1], upb, None)
            upc = upsample(acc, WIN[1], WIN[0], cfg["PHASE_A"][0], "c")
            out_r = out_p.ap().rearrange("(c p) t -> p c t", p=P)
            for ft in range(FT):
                nc.vector.tensor_tensor(
                    o[:, ft, :], o[:, ft, :], upc[:, ft, :], mybir.AluOpType.add
                )
                nc.sync.dma_start(out_r[:, ft], o[:, ft, :])

    nc.compile()
    return nc


# ---------------------------------------------------------------------------
# host-side input preparation / sharding
# ---------------------------------------------------------------------------

def make_in_maps(cfg, query, in_proj_w, in_proj_b, out_w, out_b, up_w, up_b):
    S, E, HD, F, ECH = cfg["S"], cfg["E"], cfg["HD"], cfg["F"], cfg["ECH"]
    FT = ECH
    f32 = np.float32
    f64 = np.float64

    query = np.asarray(query, f32)
    in_proj_w = np.asarray(in_proj_w, f32)
    in_proj_b = np.asarray(in_proj_b, f32)
    out_w = np.asarray(out_w, f32)
    out_b = np.asarray(out_b, f32)
    up_w = np.asarray(up_w, f32)
    up_b = np.asarray(up_b, f32)

    qT = np.ascontiguousarray(query[0].T.astype(BF16_NP))  # [E, S]

    # folded epilogue: D_l = W_out[l]^T @ up_w[l-1]^T @ ... @ up_w[0]^T
    # beta: beta_3 = b3; beta_l = beta_{l+1} @ up_w[l]^T + up_b[l] + b_l
    D = []
    for l in range(LEVELS):
        M = out_w[l].T.astype(f64)
        for j in range(l - 1, -1, -1):
            M = M @ up_w[j].T.astype(f64)
        D.append(M.astype(f32))
    Dm = np.stack(D, axis=0)  # [L, E(in), E(out)] -- already W^T layout
    beta = out_b[3].astype(f64)
    for l in range(LEVELS - 2, -1, -1):
        beta = beta @ up_w[l].T.astype(f64) + up_b[l] + out_b[l]
    beta = beta.astype(f32)

    # pack [L, e_in, e_out] -> [L, e_in%128, e_in//128, e_out//128, e_out%128]
    t = Dm.reshape(LEVELS, ECH, P, FT, P)          # [L, ec, ep, ft, fp]
    t = t.transpose(0, 2, 1, 3, 4)                 # [L, ep, ec, ft, fp]
    wd = np.ascontiguousarray(t.astype(BF16_NP))
    beta_pk = np.ascontiguousarray(beta.reshape(FT, P).T.astype(f32))  # [P, FT]

    scale = 1.0 / np.sqrt(HD).astype(f32)
    in_maps = []
    for c in range(NCORES):
        r0 = c * F
        sl_q = in_proj_w[:, r0 : r0 + F, :] * scale          # [L, F, E]
        sl_k = in_proj_w[:, E + r0 : E + r0 + F, :]
        sl_v = in_proj_w[:, 2 * E + r0 : 2 * E + r0 + F, :]
        w3 = np.stack([sl_q, sl_k, sl_v], axis=1)            # [L, 3, F, E]
        w3 = w3.transpose(0, 3, 1, 2)                        # [L, E(e), 3, F]
        w3 = w3.reshape(LEVELS, ECH, P, 3, F).transpose(0, 2, 3, 1, 4)
        w3 = np.ascontiguousarray(w3.astype(BF16_NP))        # [L, p, 3, ch, F]

        b_q = in_proj_b[:, r0 : r0 + F] * scale
        b_k = in_proj_b[:, E + r0 : E + r0 + F]
        b_v = in_proj_b[:, 2 * E + r0 : 2 * E + r0 + F]
        b3 = np.stack([b_q, b_k, b_v], axis=1)               # [L, 3, F]
        b3 = np.zeros((P, LEVELS, 3), f32) + b3.transpose(2, 0, 1)

        in_maps.append(
            {
                "qT": qT,
                "win": w3,
                "bin": np.ascontiguousarray(b3),
                "wd": wd,
                "beta": beta_pk,
            }
        )
    return in_maps


def assemble_output(cfg, results):
    S, E = cfg["S"], cfg["E"]
    blk = cfg["BLK"][0]
    out = np.empty((1, S, E), np.float32)
    for c in range(NCORES):
        out[0, c * blk : (c + 1) * blk, :] = results[c]["out"].T
    return out


_CACHE = {}


def _get_nc(cfg_key=(2048, 1024, 16)):
    if cfg_key not in _CACHE:
        cfg = _cfg(*cfg_key)
        _CACHE[cfg_key] = (cfg, build(cfg))
    return _CACHE[cfg_key]


def kernel(query, in_proj_w, in_proj_b, out_w, out_b, up_w, up_b):
    from concourse.bass_utils import run_bass_kernel_spmd

    cfg, nc = _get_nc()
    in_maps = make_in_maps(cfg, query, in_proj_w, in_proj_b, out_w, out_b, up_w, up_b)
    res = run_bass_kernel_spmd(nc, in_maps, core_ids=list(range(NCORES)))
    return assemble_output(cfg, res.results)


# revision 29
# speedup vs baseline: 1.2092x; 1.0682x over previous
"""Trainium2 Bass kernel for AdaptiveHierarchicalAttention (8 NeuronCores).

Reference computation (per level l in 0..3):
    x_l = query[:, ::2^l, :]                         # [1, S_l, E], S_l = S >> l
    outs[l] = MHA_l(x_l)                             # 16-head self-attention
Bottom-up: current = outs[3]; for l in (2,1,0):
    current = upsample_linear(current, S_l) @ up_w[l].T + up_b[l] + outs[l]

Sharding (8 cores):
  - QKV projections + attention: tensor-parallel over heads (2 heads/core).
    Scores are computed transposed (scoresT[k, q] = K @ Q^T, feature-major
    Q/K straight out of the QKV matmul), exp on ScalarE without max
    subtraction (scores are O(1) for this problem), and A = attnT^T @ V via
    an AV matmul whose lhsT is token-major V with an appended ones column,
    which yields the softmax denominator for free.
  - Per level, normalized attention outputs (feature-major, 128 feature rows
    per core) are exchanged pre-windowed per destination core: levels 1-3
    share ONE fused AllToAll issued before level-0 attention so it overlaps
    the level-0 compute, and level 0 uses an AllToAll at the end.
  - Epilogue: the up-propagation chain is LINEAR, so it is folded on the
    host: final = sum_l U_{l->0}(A_l @ D_l) + beta, with
    D_l = W_out[l]^T @ up_w[l-1]^T @ ... @ up_w[0]^T and beta the folded
    bias chain. The device multiplies each level's gathered attention
    window by D_l at the level's own (coarse) resolution, then applies the
    iterated 2x linear-interp upsampling on the Vector engine while
    accumulating level by level. Halo columns beyond the global sequence
    edges are edge-replicated in the exchange payload, reproducing the
    reference's clipped interpolation exactly. The folded D_l weights are
    prefetched during the attention phase so the epilogue is never
    DMA-bound, and levels 3..1 of the epilogue only depend on the first
    collective, so they execute in the shadow of the level-0 collective.

kernel(**inputs) takes the FULL unsharded inputs and returns the FULL output.
"""

import sys

import numpy as np

sys.path.insert(0, "/opt/trn_rl_repo")

import ml_dtypes  # noqa: E402

import concourse.mybir as mybir  # noqa: E402
import concourse.tile as tile  # noqa: E402
from concourse import bacc  # noqa: E402
from concourse.masks import make_identity  # noqa: E402

F32 = mybir.dt.float32
BF16 = mybir.dt.bfloat16
BF16_NP = ml_dtypes.bfloat16

NCORES = 8
LEVELS = 4
P = 128


def _cfg(S=2048, E=1024, H=16):
    c = {}
    c["S"], c["E"], c["H"] = S, E, H
    c["HD"] = E // H                    # head dim
    c["HPC"] = H // NCORES              # heads per core
    c["F"] = c["HPC"] * c["HD"]         # feature rows per core
    assert c["F"] == 128, "per-core feature slice must be 128"
    c["ECH"] = E // P                   # contraction chunks
    c["SL"] = [S >> l for l in range(LEVELS)]
    c["LOFF"] = np.cumsum([0] + c["SL"]).tolist()   # level offsets in token concat
    c["T"] = sum(c["SL"])               # total tokens across levels
    c["CH"] = [sl // P for sl in c["SL"]]
    c["CHOFF"] = np.cumsum([0] + c["CH"]).tolist()
    c["CHT"] = sum(c["CH"])
    c["BLK"] = [sl // NCORES for sl in c["SL"]]     # per-core token block
    # epilogue windows (token ranges incl. halos): level 0 has no halo.
    c["WIN"] = [c["BLK"][0], c["BLK"][1] + 2, c["BLK"][2] + 4, c["BLK"][3] + 4]
    # upsample phase per step l+1 -> l  (True = "even" pattern A)
    c["PHASE_A"] = [True, False, True]  # index by l of target level 0,1,2
    c["PAD"] = 2
    c["QB0"] = min(512, c["SL"][0])     # level-0 q-block width
    return c


# ---------------------------------------------------------------------------
# builder
# ---------------------------------------------------------------------------

def build(cfg, kgroup=8):
    S, E = cfg["S"], cfg["E"]
    HD, F, ECH = cfg["HD"], cfg["F"], cfg["ECH"]
    SL, LOFF, T = cfg["SL"], cfg["LOFF"], cfg["T"]
    CH, CHOFF, CHT = cfg["CH"], cfg["CHOFF"], cfg["CHT"]
    BLK, WIN, PAD = cfg["BLK"], cfg["WIN"], cfg["PAD"]
    QB0 = cfg["QB0"]
    NCK0 = SL[0] // QB0                 # number of level-0 q-blocks
    FT = ECH  # number of 128-wide feature tiles of E
    VW = 2 * HD + 4  # V-token chunk width: [V_A | 1 | pad | V_B | 1 | pad]

    nc = bacc.Bacc(
        "TRN2",
        target_bir_lowering=False,
        debug=False,
        enable_asserts=False,
        num_devices=NCORES,
    )

    # --- I/O ---------------------------------------------------------------
    qT = nc.dram_tensor("qT", [E, S], BF16, kind="ExternalInput")
    win_p = nc.dram_tensor("win", [LEVELS, P, 3, ECH, F], BF16, kind="ExternalInput")
    bin_p = nc.dram_tensor("bin", [P, LEVELS, 3], F32, kind="ExternalInput")
    # folded epilogue weights D_l (E x E each) packed for lhsT use, + beta
    wd_p = nc.dram_tensor("wd", [LEVELS, P, ECH, FT, P], BF16, kind="ExternalInput")
    beta_p = nc.dram_tensor("beta", [P, FT], F32, kind="ExternalInput")
    out_p = nc.dram_tensor("out", [E, BLK[0]], F32, kind="ExternalOutput")

    # --- internal DRAM (collective bounce) ---------------------------------
    # levels 1..3 are exchanged pre-windowed per destination: the bounce-write
    # DMA materializes per-dest overlapping window shards and one AllToAll
    # per level delivers them (levels are exchanged as soon as their
    # attention completes so only the last, smallest exchange is exposed).
    CW = [SL[3] + 2 * PAD, SL[2] + 2 * PAD, SL[1] + 2 * PAD]
    CO = {3: 0, 2: CW[0], 1: CW[0] + CW[1]}      # concat offset per level
    CTOT = sum(CW)
    HALO = {1: 1, 2: 2, 3: 2}
    # levels 3+2 share one fused exchange; level 1 gets its own (it finishes
    # last); level 0 is exchanged right after its attention so it hides under
    # level-1 compute.
    W32 = WIN[3] + WIN[2]
    agin32 = nc.dram_tensor("agin32", [NCORES, P, W32], BF16)
    g32 = nc.dram_tensor("g32", [NCORES, P, W32], BF16)
    agin1 = nc.dram_tensor("agin1", [NCORES, P, WIN[1]], BF16)
    g1 = nc.dram_tensor("g1", [NCORES, P, WIN[1]], BF16)
    # level 0 is exchanged as two half-exchanges over strided q-sets (each
    # carries the first/second 128-token half of every dest's block), so the
    # first one hides under the second half of level-0 attention.
    HB = BLK[0] // 2
    agin0a = nc.dram_tensor("agin0a", [NCORES, P, HB], BF16)
    g0a = nc.dram_tensor("g0a", [NCORES, P, HB], BF16)
    agin0b = nc.dram_tensor("agin0b", [NCORES, P, HB], BF16)
    g0b = nc.dram_tensor("g0b", [NCORES, P, HB], BF16)
    rg = [list(range(NCORES))]

    with tile.TileContext(nc) as tc:
        from contextlib import ExitStack

        with ExitStack() as ctx:
            pool = lambda name, bufs, **kw: ctx.enter_context(
                tc.tile_pool(name=name, bufs=bufs, **kw)
            )
            const = pool("const", 1)
            # folded epilogue weights live for the whole kernel (prefetched
            # during attention); the epilogue work pools are created after
            # the attention pools close and reuse their SBUF/PSUM space.
            d_pool = pool("dw", 1)
            g_pool = pool("gpool", 1)

            stackA = ctx.enter_context(ExitStack())
            poolA = lambda name, bufs, **kw: stackA.enter_context(
                tc.tile_pool(name=name, bufs=bufs, **kw)
            )
            qk_pool = poolA("qk", 1)
            vf_pool = poolA("vf", 1)
            at_pool = poolA("at", 8)
            nrm_pool = poolA("nrm", 2)
            a0_pool = poolA("a0", 2)
            qkv_ps = poolA("qkv_ps", 1, space="PSUM")
            tr_ps = poolA("tr_ps", 1, space="PSUM")
            sc_ps = poolA("sc_ps", 2, space="PSUM")
            av_ps = poolA("av_ps", 2, space="PSUM")

            # --- constants / persistent buffers ---------------------------
            b_sb = const.tile([P, LEVELS, 3], F32, tag="b_sb")
            nc.sync.dma_start(b_sb[:], bin_p[:])
            beta_sb = const.tile([P, FT], F32, tag="beta_sb")
            nc.sync.dma_start(beta_sb[:], beta_p[:])

            ident = const.tile([P, P], BF16, tag="ident")
            make_identity(nc, ident[:])
            ones_sb = const.tile([P, HD], BF16, tag="ones")
            nc.vector.memset(ones_sb[:], 1.0)

            # QKV weights for all levels (persistent; level-3 slice first so
            # compute can start ASAP, then the query transpose, then the rest)
            wl_sb = const.tile([P, LEVELS, 3, ECH, F], BF16, tag="wl_sb")
            win_r = win_p.ap().rearrange("l p three c f -> p l three c f")
            nc.sync.dma_start(wl_sb[:, 3], win_r[:, 3])

            xT = qk_pool.tile([P, ECH, S], BF16, tag="xT")
            qT_r = qT.ap().rearrange("(c p) t -> p c t", p=P)
            for c in range(ECH):
                nc.sync.dma_start(xT[:, c, :], qT_r[:, c, :])
            for l in (2, 0, 1):
                nc.sync.dma_start(wl_sb[:, l], win_r[:, l])

            # prefetch folded epilogue weights in per-chunk pieces (keeps
            # head-of-line blocking on the DMA queue small)
            wd_sb = d_pool.tile([P, LEVELS, ECH, FT, P], BF16, tag="wd_sb")
            wd_r = wd_p.ap().rearrange("l p c ft f -> p l c ft f")
            for l in (3, 2, 1, 0):
                for c in range(ECH):
                    nc.sync.dma_start(wd_sb[:, l, c], wd_r[:, l, c])

            Q = qk_pool.tile([P, T], BF16, tag="Q")
            K = qk_pool.tile([P, T], BF16, tag="K")
            Vt = qk_pool.tile([P, CHT, VW], BF16, tag="Vt")
            nc.vector.memset(Vt[:, :, HD : HD + 1], 1.0)
            nc.vector.memset(Vt[:, :, 2 * HD + 2 : 2 * HD + 3], 1.0)

            # ---------------- per-level QKV + attention -------------------
            # QKV work is emitted as a queue of small closures so attention
            # blocks can drain it into their PE bubbles (PE waits on ScalarE
            # exps between the score and AV matmuls of a block).
            def qkv_chunks(l):
                stride = 1 << l
                sl = SL[l]
                nt = min(512, sl)
                vfeat = vf_pool.tile([F, SL[0]], BF16, tag="vf", name=f"vf{l}")

                def proj(part, n0, dst):
                    def emit():
                        ps = qkv_ps.tile([F, nt], F32, tag="qkv", name="qkvps")
                        for c in range(ECH):
                            rhs = xT[:, c, n0 * stride : (n0 + nt) * stride : stride]
                            nc.tensor.matmul(
                                ps[:],
                                lhsT=wl_sb[:, l, part, c, :],
                                rhs=rhs,
                                start=(c == 0),
                                stop=(c == ECH - 1),
                            )
                        if part < 2:
                            o = dst[:, LOFF[l] + n0 : LOFF[l] + n0 + nt]
                        else:
                            o = dst[:, n0 : n0 + nt]
                        nc.vector.tensor_tensor(
                            o,
                            ps[:],
                            b_sb[:, l, part : part + 1].to_broadcast((F, nt)),
                            mybir.AluOpType.add,
                        )

                    return emit

                def vtrans(j):
                    def emit():
                        tp = tr_ps.tile([P, F], BF16, tag="tr", name="trps")
                        nc.tensor.transpose(
                            tp[:], vfeat[:, j * P : (j + 1) * P], ident[:F, :F]
                        )
                        ch = CHOFF[l] + j
                        nc.vector.tensor_copy(out=Vt[:, ch, 0:HD], in_=tp[:, 0:HD])
                        nc.vector.tensor_copy(
                            out=Vt[:, ch, HD + 2 : 2 * HD + 2], in_=tp[:, HD : 2 * HD]
                        )

                    return emit

                work = []
                for part, dst in ((0, Q), (1, K), (2, vfeat)):
                    for n0 in range(0, sl, nt):
                        work.append(proj(part, n0, dst))
                for j in range(CH[l]):
                    work.append(vtrans(j))
                return work

            def drain(work, n=None):
                k = len(work) if n is None else min(n, len(work))
                for _ in range(k):
                    work.pop(0)()

            def attn_block(l, qb0, qbw, a_dst, a_off, filler=None, qstride=None):
                """Attention for q-block [qb0, qb0+qbw) of level l -> a_dst[:, a_off:].

                With qstride, the q-set is qbw//128 chunks of 128 tokens at
                stride qstride starting at qb0 (strided level-0 half-blocks).
                """
                nch = CH[l]

                def qrhs(b):
                    if qstride is None:
                        return Q[b : b + HD, LOFF[l] + qb0 : LOFF[l] + qb0 + qbw]
                    r = Q[b : b + HD, LOFF[l] + qb0 : LOFF[l] + qb0 + P]
                    r.ap.insert(1, [qstride, qbw // P])
                    return r
                avA = av_ps.tile([HD + 1, qbw], F32, tag="av")
                avB = av_ps.tile([HD + 1, qbw], F32, tag="av")
                for g0_ in range(0, nch, kgroup):
                    gch = list(range(g0_, min(g0_ + kgroup, nch)))
                    ats = {}
                    # score chunks in pairs: one 2-bank PSUM tile, one exp
                    # instruction per pair (amortizes ScalarE per-op cost)
                    for i0 in range(0, len(gch), 2):
                        pair = gch[i0 : i0 + 2]
                        for h in (0, 1):
                            b = h * HD
                            sp = sc_ps.tile([P, 2 * qbw], F32, tag="sc")
                            for j, kc in enumerate(pair):
                                nc.tensor.matmul(
                                    sp[:, j * qbw : (j + 1) * qbw],
                                    lhsT=K[b : b + HD, LOFF[l] + kc * P : LOFF[l] + (kc + 1) * P],
                                    rhs=qrhs(b),
                                    start=True,
                                    stop=True,
                                )
                            at = at_pool.tile([P, 2 * qbw], BF16, tag="at")
                            nc.scalar.activation(
                                at[:, 0 : len(pair) * qbw],
                                sp[:, 0 : len(pair) * qbw],
                                mybir.ActivationFunctionType.Exp,
                            )
                            for j, kc in enumerate(pair):
                                ats[(kc, h)] = at[:, j * qbw : (j + 1) * qbw]
                    if filler is not None:
                        filler()
                    for kc in gch:
                        for h, av in ((0, avA), (1, avB)):
                            c0 = 0 if h == 0 else HD + 2
                            nc.tensor.matmul(
                                av[:],
                                lhsT=Vt[:, CHOFF[l] + kc, c0 : c0 + HD + 1],
                                rhs=ats[(kc, h)],
                                start=(kc == 0),
                                stop=(kc == nch - 1),
                            )
                    if filler is not None:
                        filler()

                def _norm_bc(av):
                    dn = nrm_pool.tile([P, qbw], BF16, tag="dn")
                    nc.vector.tensor_copy(out=dn[HD : HD + 1, :], in_=av[HD : HD + 1, :])
                    with nc.allow_low_precision(
                        reason="softmax denominators tolerate bf16 recip"
                    ):
                        nc.vector.reciprocal(dn[HD : HD + 1, :], dn[HD : HD + 1, :])
                    bc_ps = tr_ps.tile([HD, qbw], F32, tag="tr")
                    nc.tensor.matmul(
                        bc_ps[:],
                        lhsT=ones_sb[HD : HD + 1, 0:HD],
                        rhs=dn[HD : HD + 1, :],
                        start=True,
                        stop=True,
                    )
                    bc = nrm_pool.tile([HD, qbw], F32, tag="bc_sb")
                    nc.vector.tensor_copy(out=bc[:], in_=bc_ps[:])
                    return bc

                bcA = _norm_bc(avA)
                nc.vector.tensor_mul(
                    out=a_dst[0:HD, a_off : a_off + qbw], in0=avA[0:HD, :], in1=bcA[:]
                )
                bcB = _norm_bc(avB)
                tmpB = nrm_pool.tile([HD, qbw], BF16, tag="tmpB")
                nc.vector.tensor_mul(out=tmpB[:], in0=avB[0:HD, :], in1=bcB[:])
                # head B rows live at partitions HD..2HD: shift via DMA
                nc.sync.dma_start(a_dst[HD : 2 * HD, a_off : a_off + qbw], tmpB[:])

            A123 = qk_pool.tile([P, CTOT], BF16, tag="A123")

            def attn_level_whole(l, work, cap=4):
                """Levels 1..3: write into the fused concat buffer (padded)."""
                sl = SL[l]
                co = CO[l]
                qbw = min(512, sl)
                nblk = sl // qbw
                calls = 2 * nblk * -(-CH[l] // kgroup)
                per = min(cap, -(-len(work) // calls)) if work else 0
                for qb0 in range(0, sl, qbw):
                    attn_block(
                        l, qb0, qbw, A123, co + PAD + qb0,
                        filler=lambda: drain(work, per),
                    )
                nc.vector.tensor_copy(
                    out=A123[:, co : co + PAD],
                    in_=A123[:, co + PAD : co + PAD + 1].to_broadcast((P, PAD)),
                )
                nc.vector.tensor_copy(
                    out=A123[:, co + PAD + sl : co + 2 * PAD + sl],
                    in_=A123[:, co + PAD + sl - 1 : co + PAD + sl].to_broadcast((P, PAD)),
                )

            def bounce_windows(l, dst_dram, woff):
                """One DMA materializing all 8 overlapping dest windows."""
                s0 = CO[l] + PAD - HALO[l]
                src = A123[:, s0 : s0 + WIN[l]]
                src.ap.insert(1, [BLK[l], NCORES])
                dst = dst_dram.ap().rearrange("d p w -> p d w")
                nc.sync.dma_start(dst[:, :, woff : woff + WIN[l]], src)

            def a2a(ins_t, outs_t):
                nc.gpsimd.collective_compute(
                    "AllToAll",
                    mybir.AluOpType.bypass,
                    replica_groups=rg,
                    ins=[ins_t[:]],
                    outs=[outs_t[:]],
                )

            def attn_level0_pass(work, half, ag, gout):
                """One strided half-pass of level 0: the q-set is the
                `half`-th 128-token half of every dest core's 256-block.
                Two 512-wide blocks (dests 0-3, dests 4-7), then bounce+a2a."""
                calls = 2 * NCK0 // 2 * -(-CH[0] // kgroup)
                per = min(4, -(-len(work) // calls)) if work else 0
                for b in range(2):
                    A0 = a0_pool.tile([P, QB0], BF16, tag="A0")
                    attn_block(
                        0,
                        b * 4 * BLK[0] + half * HB,
                        QB0,
                        A0,
                        0,
                        filler=lambda: drain(work, per),
                        qstride=BLK[0],
                    )
                    # 4 dests x 128 cols per block
                    nc.sync.dma_start(
                        ag.ap()[b * 4 : (b + 1) * 4].rearrange("d p w -> p d w"),
                        A0[:].rearrange("p (d w) -> p d w", d=4),
                    )
                a2a(ag, gout)

            # ---------------- epilogue ------------------------------------
            # Z_l = (gathered A_l window) @ D_l at level-l resolution, then
            # chained 2x upsample + accumulate on DVE:
            #   acc_3 = Z_3; acc_l = U(acc_{l+1}) + Z_l; out = acc_0 + beta
            def z_level(gtile, goff, l, w):
                """Matmul Z_l -> list of psum tiles (one per ft)."""
                zt = acc_pool.tile([P, FT, w], BF16, tag=f"z{l}")
                for ft in range(FT):
                    ps = ep_ps.tile([P, w], F32, tag="ep")
                    for c in range(ECH):
                        nc.tensor.matmul(
                            ps[:],
                            lhsT=wd_sb[:, l, c, ft],
                            rhs=gtile[:, c, goff : goff + w],
                            start=(c == 0),
                            stop=(c == ECH - 1),
                        )
                    nc.vector.tensor_copy(out=zt[:, ft, :], in_=ps[:])
                return zt

            def z_level_add(gtile, goff, l, w, up, extra):
                """Z_l matmuls, then out_tile = psum + up (+ extra bias)."""
                res = acc_pool.tile([P, FT, w], F32 if l == 0 else BF16, tag=f"acc{l}")
                for ft in range(FT):
                    ps = ep_ps.tile([P, w], F32, tag="ep")
                    for c in range(ECH):
                        nc.tensor.matmul(
                            ps[:],
                            lhsT=wd_sb[:, l, c, ft],
                            rhs=gtile[:, c, goff : goff + w],
                            start=(c == 0),
                            stop=(c == ECH - 1),
                        )
                    nc.vector.tensor_tensor(
                        res[:, ft, :], ps[:], up[:, ft, :], mybir.AluOpType.add
                    )
                    if extra is not None:
                        nc.vector.tensor_tensor(
                            res[:, ft, :],
                            res[:, ft, :],
                            extra[:, ft : ft + 1].to_broadcast((P, w)),
                            mybir.AluOpType.add,
                        )
                return res

            def upsample(cur, ws, w, phase_a, tag):
                """2x linear-interp upsample [P, FT, ws] -> [P, FT, w] (DVE)."""
                p25 = acc_pool.tile([P, FT, ws], BF16, tag=f"p25{tag}")
                p75 = acc_pool.tile([P, FT, ws], BF16, tag=f"p75{tag}")
                nc.vector.tensor_scalar_mul(p25[:], cur[:], 0.25)
                nc.vector.tensor_scalar_mul(p75[:], cur[:], 0.75)
                up = acc_pool.tile([P, FT, w], BF16, tag=f"up{tag}")
                hw = (w + 1) // 2
                hw2 = w // 2
                if phase_a:
                    nc.vector.tensor_add(
                        up[:, :, 0::2], p25[:, :, 0:hw], p75[:, :, 1 : hw + 1]
                    )
                    nc.vector.tensor_add(
                        up[:, :, 1::2], p75[:, :, 1 : hw2 + 1], p25[:, :, 2 : hw2 + 2]
                    )
                else:
                    nc.vector.tensor_add(
                        up[:, :, 0::2], p75[:, :, 1 : hw + 1], p25[:, :, 2 : hw + 2]
                    )
                    nc.vector.tensor_add(
                        up[:, :, 1::2], p25[:, :, 1 : hw2 + 1], p75[:, :, 2 : hw2 + 2]
                    )
                return up

            # ---------------- schedule ------------------------------------
            # level order 3, 2, 1, 0: the fused 3+2 exchange and the level-1
            # exchange go out during attention; level 0 runs last as two
            # strided half-passes so its first half-exchange hides under the
            # second half-pass and only the second (small) one is exposed,
            # followed by just half of Z_0 + final adds.
            drain(qkv_chunks(3))
            w2 = qkv_chunks(2)
            w1 = qkv_chunks(1)
            w0 = qkv_chunks(0)
            attn_level_whole(3, w2, cap=5)
            drain(w2)
            attn_level_whole(2, w1, cap=7)
            bounce_windows(3, agin32, 0)
            bounce_windows(2, agin32, WIN[3])
            a2a(agin32, g32)
            Gs32 = g_pool.tile([P, ECH, W32], BF16, tag="gs32")
            nc.sync.dma_start(Gs32[:], g32.ap().rearrange("b p t -> p b t"))
            drain(w1)
            attn_level_whole(1, w0, cap=6)
            bounce_windows(1, agin1, 0)
            a2a(agin1, g1)
            Gs1 = g_pool.tile([P, ECH, WIN[1]], BF16, tag="gs1")
            nc.sync.dma_start(Gs1[:], g1.ap().rearrange("b p t -> p b t"))
            drain(w0)
            attn_level0_pass([], 0, agin0a, g0a)
            attn_level0_pass([], 1, agin0b, g0b)

            stackA.close()
            acc_pool = ctx.enter_context(tc.tile_pool(name="acc", bufs=1))
            ep_ps = ctx.enter_context(tc.tile_pool(name="ep_ps", bufs=2, space="PSUM"))

            # chain part that needs only g32/g1 (both landed during attention)
            acc = z_level(Gs32, 0, 3, WIN[3])
            up = upsample(acc, WIN[3], WIN[2], cfg["PHASE_A"][2], "a")
            acc = z_level_add(Gs32, WIN[3], 2, WIN[2], up, None)
            upb = upsample(acc, WIN[2], WIN[1], cfg["PHASE_A"][1], "b")
            acc = z_level_add(Gs1, 0, 1, WIN[1], upb, None)
            upc = upsample(acc, WIN[1], WIN[0], cfg["PHASE_A"][0], "c")

            # level-0 halves: matmul + (beta, upc) adds + streamed out DMA
            out_r = out_p.ap().rearrange("(c p) t -> p c t", p=P)
            for half, gsrc in ((0, g0a), (1, g0b)):
                gs = g_pool.tile(
                    [P, ECH, HB], BF16, tag=f"gs0{half}", name=f"gs0{half}"
                )
                nc.sync.dma_start(gs[:], gsrc.ap().rearrange("b p t -> p b t"))
                for ft in range(FT):
                    ps = ep_ps.tile([P, HB], F32, tag="ep")
                    for c in range(ECH):
                        nc.tensor.matmul(
                            ps[:],
                            lhsT=wd_sb[:, 0, c, ft],
                            rhs=gs[:, c, :],
                            start=(c == 0),
                            stop=(c == ECH - 1),
                        )
                    o = acc_pool.tile([P, HB], F32, tag=f"o{half}_{ft}", name="o_t")
                    nc.vector.tensor_tensor(
                        o[:],
                        ps[:],
                        beta_sb[:, ft : ft + 1].to_broadcast((P, HB)),
                        mybir.AluOpType.add,
                    )
                    nc.vector.tensor_tensor(
                        o[:],
                        o[:],
                        upc[:, ft, half * HB : (half + 1) * HB],
                        mybir.AluOpType.add,
                    )
                    nc.sync.dma_start(
                        out_r[:, ft, half * HB : (half + 1) * HB], o[:]
                    )

    nc.compile()
    return nc


# ---------------------------------------------------------------------------
# host-side input preparation / sharding
# ---------------------------------------------------------------------------

def make_in_maps(cfg, query, in_proj_w, in_proj_b, out_w, out_b, up_w, up_b):
    S, E, HD, F, ECH = cfg["S"], cfg["E"], cfg["HD"], cfg["F"], cfg["ECH"]
    FT = ECH
    f32 = np.float32
    f64 = np.float64

    query = np.asarray(query, f32)
    in_proj_w = np.asarray(in_proj_w, f32)
    in_proj_b = np.asarray(in_proj_b, f32)
    out_w = np.asarray(out_w, f32)
    out_b = np.asarray(out_b, f32)
    up_w = np.asarray(up_w, f32)
    up_b = np.asarray(up_b, f32)

    qT = np.ascontiguousarray(query[0].T.astype(BF16_NP))  # [E, S]

    # folded epilogue: D_l = W_out[l]^T @ up_w[l-1]^T @ ... @ up_w[0]^T
    # beta: beta_3 = b3; beta_l = beta_{l+1} @ up_w[l]^T + up_b[l] + b_l
    D = []
    for l in range(LEVELS):
        M = out_w[l].T.astype(f64)
        for j in range(l - 1, -1, -1):
            M = M @ up_w[j].T.astype(f64)
        D.append(M.astype(f32))
    Dm = np.stack(D, axis=0)  # [L, E(in), E(out)] -- already W^T layout
    beta = out_b[3].astype(f64)
    for l in range(LEVELS - 2, -1, -1):
        beta = beta @ up_w[l].T.astype(f64) + up_b[l] + out_b[l]
    beta = beta.astype(f32)

    # pack [L, e_in, e_out] -> [L, e_in%128, e_in//128, e_out//128, e_out%128]
    t = Dm.reshape(LEVELS, ECH, P, FT, P)          # [L, ec, ep, ft, fp]
    t = t.transpose(0, 2, 1, 3, 4)                 # [L, ep, ec, ft, fp]
    wd = np.ascontiguousarray(t.astype(BF16_NP))
    beta_pk = np.ascontiguousarray(beta.reshape(FT, P).T.astype(f32))  # [P, FT]

    scale = 1.0 / np.sqrt(HD).astype(f32)
    in_maps = []
    for c in range(NCORES):
        r0 = c * F
        sl_q = in_proj_w[:, r0 : r0 + F, :] * scale          # [L, F, E]
        sl_k = in_proj_w[:, E + r0 : E + r0 + F, :]
        sl_v = in_proj_w[:, 2 * E + r0 : 2 * E + r0 + F, :]
        w3 = np.stack([sl_q, sl_k, sl_v], axis=1)            # [L, 3, F, E]
        w3 = w3.transpose(0, 3, 1, 2)                        # [L, E(e), 3, F]
        w3 = w3.reshape(LEVELS, ECH, P, 3, F).transpose(0, 2, 3, 1, 4)
        w3 = np.ascontiguousarray(w3.astype(BF16_NP))        # [L, p, 3, ch, F]

        b_q = in_proj_b[:, r0 : r0 + F] * scale
        b_k = in_proj_b[:, E + r0 : E + r0 + F]
        b_v = in_proj_b[:, 2 * E + r0 : 2 * E + r0 + F]
        b3 = np.stack([b_q, b_k, b_v], axis=1)               # [L, 3, F]
        b3 = np.zeros((P, LEVELS, 3), f32) + b3.transpose(2, 0, 1)

        in_maps.append(
            {
                "qT": qT,
                "win": w3,
                "bin": np.ascontiguousarray(b3),
                "wd": wd,
                "beta": beta_pk,
            }
        )
    return in_maps


def assemble_output(cfg, results):
    S, E = cfg["S"], cfg["E"]
    blk = cfg["BLK"][0]
    out = np.empty((1, S, E), np.float32)
    for c in range(NCORES):
        out[0, c * blk : (c + 1) * blk, :] = results[c]["out"].T
    return out


_CACHE = {}


def _get_nc(cfg_key=(2048, 1024, 16)):
    if cfg_key not in _CACHE:
        cfg = _cfg(*cfg_key)
        _CACHE[cfg_key] = (cfg, build(cfg))
    return _CACHE[cfg_key]


def kernel(query, in_proj_w, in_proj_b, out_w, out_b, up_w, up_b):
    from concourse.bass_utils import run_bass_kernel_spmd

    cfg, nc = _get_nc()
    in_maps = make_in_maps(cfg, query, in_proj_w, in_proj_b, out_w, out_b, up_w, up_b)
    res = run_bass_kernel_spmd(nc, in_maps, core_ids=list(range(NCORES)))
    return assemble_output(cfg, res.results)


# revision 35
# speedup vs baseline: 1.2617x; 1.0434x over previous
"""Trainium2 Bass kernel for AdaptiveHierarchicalAttention (8 NeuronCores).

Reference computation (per level l in 0..3):
    x_l = query[:, ::2^l, :]                         # [1, S_l, E], S_l = S >> l
    outs[l] = MHA_l(x_l)                             # 16-head self-attention
Bottom-up: current = outs[3]; for l in (2,1,0):
    current = upsample_linear(current, S_l) @ up_w[l].T + up_b[l] + outs[l]

Sharding (8 cores):
  - QKV projections + attention: tensor-parallel over heads (2 heads/core).
    Scores are computed transposed (scoresT[k, q] = K @ Q^T, feature-major
    Q/K straight out of the QKV matmul), exp on ScalarE without max
    subtraction (scores are O(1) for this problem), and A = attnT^T @ V via
    an AV matmul whose lhsT is token-major V with an appended ones column,
    which yields the softmax denominator for free.
  - Per level, normalized attention outputs (feature-major, 128 feature rows
    per core) are exchanged pre-windowed per destination core: levels 1-3
    share ONE fused AllToAll issued before level-0 attention so it overlaps
    the level-0 compute, and level 0 uses an AllToAll at the end.
  - Epilogue: the up-propagation chain is LINEAR, so it is folded on the
    host: final = sum_l U_{l->0}(A_l @ D_l) + beta, with
    D_l = W_out[l]^T @ up_w[l-1]^T @ ... @ up_w[0]^T and beta the folded
    bias chain. The device multiplies each level's gathered attention
    window by D_l at the level's own (coarse) resolution, then applies the
    iterated 2x linear-interp upsampling on the Vector engine while
    accumulating level by level. Halo columns beyond the global sequence
    edges are edge-replicated in the exchange payload, reproducing the
    reference's clipped interpolation exactly. The folded D_l weights are
    prefetched during the attention phase so the epilogue is never
    DMA-bound, and levels 3..1 of the epilogue only depend on the first
    collective, so they execute in the shadow of the level-0 collective.

kernel(**inputs) takes the FULL unsharded inputs and returns the FULL output.
"""

import sys

import numpy as np

sys.path.insert(0, "/opt/trn_rl_repo")

import ml_dtypes  # noqa: E402

import concourse.mybir as mybir  # noqa: E402
import concourse.tile as tile  # noqa: E402
from concourse import bacc  # noqa: E402
from concourse.masks import make_identity  # noqa: E402

F32 = mybir.dt.float32
BF16 = mybir.dt.bfloat16
BF16_NP = ml_dtypes.bfloat16

NCORES = 8
LEVELS = 4
P = 128


def _cfg(S=2048, E=1024, H=16):
    c = {}
    c["S"], c["E"], c["H"] = S, E, H
    c["HD"] = E // H                    # head dim
    c["HPC"] = H // NCORES              # heads per core
    c["F"] = c["HPC"] * c["HD"]         # feature rows per core
    assert c["F"] == 128, "per-core feature slice must be 128"
    c["ECH"] = E // P                   # contraction chunks
    c["SL"] = [S >> l for l in range(LEVELS)]
    c["LOFF"] = np.cumsum([0] + c["SL"]).tolist()   # level offsets in token concat
    c["T"] = sum(c["SL"])               # total tokens across levels
    c["CH"] = [sl // P for sl in c["SL"]]
    c["CHOFF"] = np.cumsum([0] + c["CH"]).tolist()
    c["CHT"] = sum(c["CH"])
    c["BLK"] = [sl // NCORES for sl in c["SL"]]     # per-core token block
    # epilogue windows (token ranges incl. halos): level 0 has no halo.
    c["WIN"] = [c["BLK"][0], c["BLK"][1] + 2, c["BLK"][2] + 4, c["BLK"][3] + 4]
    # upsample phase per step l+1 -> l  (True = "even" pattern A)
    c["PHASE_A"] = [True, False, True]  # index by l of target level 0,1,2
    c["PAD"] = 2
    c["QB0"] = min(512, c["SL"][0])     # level-0 q-block width
    return c


# ---------------------------------------------------------------------------
# builder
# ---------------------------------------------------------------------------

def build(cfg, kgroup=8):
    S, E = cfg["S"], cfg["E"]
    HD, F, ECH = cfg["HD"], cfg["F"], cfg["ECH"]
    SL, LOFF, T = cfg["SL"], cfg["LOFF"], cfg["T"]
    CH, CHOFF, CHT = cfg["CH"], cfg["CHOFF"], cfg["CHT"]
    BLK, WIN, PAD = cfg["BLK"], cfg["WIN"], cfg["PAD"]
    QB0 = cfg["QB0"]
    NCK0 = SL[0] // QB0                 # number of level-0 q-blocks
    FT = ECH  # number of 128-wide feature tiles of E
    VW = 2 * HD + 4  # V-token chunk width: [V_A | 1 | pad | V_B | 1 | pad]

    nc = bacc.Bacc(
        "TRN2",
        target_bir_lowering=False,
        debug=False,
        enable_asserts=False,
        num_devices=NCORES,
    )

    # --- I/O ---------------------------------------------------------------
    qT = nc.dram_tensor("qT", [E, S], BF16, kind="ExternalInput")
    win_p = nc.dram_tensor("win", [LEVELS, P, 3, ECH, F], BF16, kind="ExternalInput")
    bin_p = nc.dram_tensor("bin", [P, LEVELS, 3], F32, kind="ExternalInput")
    # folded epilogue weights D_l (E x E each) packed for lhsT use, + beta
    wd_p = nc.dram_tensor("wd", [LEVELS, P, ECH, FT, P], BF16, kind="ExternalInput")
    beta_p = nc.dram_tensor("beta", [P, FT], F32, kind="ExternalInput")
    out_p = nc.dram_tensor("out", [E, BLK[0]], F32, kind="ExternalOutput")

    # --- internal DRAM (collective bounce) ---------------------------------
    # levels 1..3 are exchanged pre-windowed per destination: the bounce-write
    # DMA materializes per-dest overlapping window shards and one AllToAll
    # per level delivers them (levels are exchanged as soon as their
    # attention completes so only the last, smallest exchange is exposed).
    CW = [SL[3] + 2 * PAD, SL[2] + 2 * PAD, SL[1] + 2 * PAD]
    CO = {3: 0, 2: CW[0], 1: CW[0] + CW[1]}      # concat offset per level
    CTOT = sum(CW)
    HALO = {1: 1, 2: 2, 3: 2}
    # levels 3+2 share one fused exchange; level 1 gets its own (it finishes
    # last); level 0 is exchanged right after its attention so it hides under
    # level-1 compute.
    W32 = WIN[3] + WIN[2]
    agin32 = nc.dram_tensor("agin32", [NCORES, P, W32], BF16)
    g32 = nc.dram_tensor("g32", [NCORES, P, W32], BF16)
    agin1 = nc.dram_tensor("agin1", [NCORES, P, WIN[1]], BF16)
    g1 = nc.dram_tensor("g1", [NCORES, P, WIN[1]], BF16)
    # level 0 is exchanged as two half-exchanges over strided q-sets (each
    # carries the first/second 128-token half of every dest's block), so the
    # first one hides under the second half of level-0 attention.
    HB = BLK[0] // 2
    agin0a = nc.dram_tensor("agin0a", [NCORES, P, HB], BF16)
    g0a = nc.dram_tensor("g0a", [NCORES, P, HB], BF16)
    agin0b = nc.dram_tensor("agin0b", [NCORES, P, HB], BF16)
    g0b = nc.dram_tensor("g0b", [NCORES, P, HB], BF16)
    rg = [list(range(NCORES))]

    with tile.TileContext(nc) as tc:
        from contextlib import ExitStack

        with ExitStack() as ctx:
            pool = lambda name, bufs, **kw: ctx.enter_context(
                tc.tile_pool(name=name, bufs=bufs, **kw)
            )
            const = pool("const", 1)
            # folded epilogue weights live for the whole kernel (prefetched
            # during attention); the epilogue work pools are created after
            # the attention pools close and reuse their SBUF/PSUM space.
            d_pool = pool("dw", 1)
            g_pool = pool("gpool", 1)

            stackA = ctx.enter_context(ExitStack())
            poolA = lambda name, bufs, **kw: stackA.enter_context(
                tc.tile_pool(name=name, bufs=bufs, **kw)
            )
            qk_pool = poolA("qk", 1)
            vf_pool = poolA("vf", 1)
            at_pool = poolA("at", 8)
            nrm_pool = poolA("nrm", 2)
            a0_pool = poolA("a0", 2)
            qkv_ps = poolA("qkv_ps", 1, space="PSUM")
            tr_ps = poolA("tr_ps", 1, space="PSUM")
            sc_ps = poolA("sc_ps", 2, space="PSUM")
            av_ps = poolA("av_ps", 1, space="PSUM")

            # --- constants / persistent buffers ---------------------------
            b_sb = const.tile([P, LEVELS, 3], F32, tag="b_sb")
            nc.sync.dma_start(b_sb[:], bin_p[:])
            beta_sb = const.tile([P, FT], F32, tag="beta_sb")
            nc.sync.dma_start(beta_sb[:], beta_p[:])

            ident = const.tile([P, P], BF16, tag="ident")
            make_identity(nc, ident[:])

            # QKV weights for all levels (persistent; level-3 slice first so
            # compute can start ASAP, then the query transpose, then the rest)
            wl_sb = const.tile([P, LEVELS, 3, ECH, F], BF16, tag="wl_sb")
            win_r = win_p.ap().rearrange("l p three c f -> p l three c f")
            nc.sync.dma_start(wl_sb[:, 3], win_r[:, 3])

            xT = qk_pool.tile([P, ECH, S], BF16, tag="xT")
            qT_r = qT.ap().rearrange("(c p) t -> p c t", p=P)
            for c in range(ECH):
                nc.sync.dma_start(xT[:, c, :], qT_r[:, c, :])
            for l in (2, 0, 1):
                nc.sync.dma_start(wl_sb[:, l], win_r[:, l])

            # prefetch folded epilogue weights in per-chunk pieces (keeps
            # head-of-line blocking on the DMA queue small)
            wd_sb = d_pool.tile([P, LEVELS, ECH, FT, P], BF16, tag="wd_sb")
            wd_r = wd_p.ap().rearrange("l p c ft f -> p l c ft f")
            for l in (3, 2, 1, 0):
                for c in range(ECH):
                    nc.sync.dma_start(wd_sb[:, l, c], wd_r[:, l, c])

            Q = qk_pool.tile([P, T], BF16, tag="Q")
            K = qk_pool.tile([P, T], BF16, tag="K")
            Vt = qk_pool.tile([P, CHT, VW], BF16, tag="Vt")
            nc.vector.memset(Vt[:, :, HD : HD + 1], 1.0)
            nc.vector.memset(Vt[:, :, 2 * HD + 2 : 2 * HD + 3], 1.0)

            # ---------------- per-level QKV + attention -------------------
            # QKV work is emitted as a queue of small closures so attention
            # blocks can drain it into their PE bubbles (PE waits on ScalarE
            # exps between the score and AV matmuls of a block).
            def qkv_chunks(l):
                stride = 1 << l
                sl = SL[l]
                nt = min(512, sl)
                vfeat = vf_pool.tile([F, SL[0]], BF16, tag="vf", name=f"vf{l}")

                def proj(part, n0, dst):
                    def emit():
                        ps = qkv_ps.tile([F, nt], F32, tag="qkv", name="qkvps")
                        for c in range(ECH):
                            rhs = xT[:, c, n0 * stride : (n0 + nt) * stride : stride]
                            nc.tensor.matmul(
                                ps[:],
                                lhsT=wl_sb[:, l, part, c, :],
                                rhs=rhs,
                                start=(c == 0),
                                stop=(c == ECH - 1),
                            )
                        if part < 2:
                            o = dst[:, LOFF[l] + n0 : LOFF[l] + n0 + nt]
                        else:
                            o = dst[:, n0 : n0 + nt]
                        nc.vector.tensor_tensor(
                            o,
                            ps[:],
                            b_sb[:, l, part : part + 1].to_broadcast((F, nt)),
                            mybir.AluOpType.add,
                        )

                    return emit

                def vtrans(j):
                    def emit():
                        tp = tr_ps.tile([P, F], BF16, tag="tr", name="trps")
                        nc.tensor.transpose(
                            tp[:], vfeat[:, j * P : (j + 1) * P], ident[:F, :F]
                        )
                        ch = CHOFF[l] + j
                        nc.vector.tensor_copy(out=Vt[:, ch, 0:HD], in_=tp[:, 0:HD])
                        nc.vector.tensor_copy(
                            out=Vt[:, ch, HD + 2 : 2 * HD + 2], in_=tp[:, HD : 2 * HD]
                        )

                    return emit

                work = []
                for part, dst in ((0, Q), (1, K), (2, vfeat)):
                    for n0 in range(0, sl, nt):
                        work.append(proj(part, n0, dst))
                for j in range(CH[l]):
                    work.append(vtrans(j))
                return work

            def drain(work, n=None):
                k = len(work) if n is None else min(n, len(work))
                for _ in range(k):
                    work.pop(0)()

            def attn_block(l, qb0, qbw, a_dst, a_off, filler=None, qstride=None):
                """Attention for q-block [qb0, qb0+qbw) of level l -> a_dst[:, a_off:].

                With qstride, the q-set is qbw//128 chunks of 128 tokens at
                stride qstride starting at qb0 (strided level-0 half-blocks).
                """
                nch = CH[l]
                nqc = qbw // P

                def qrhs(b):
                    if qstride is None:
                        return Q[b : b + HD, LOFF[l] + qb0 : LOFF[l] + qb0 + qbw]
                    r = Q[b : b + HD, LOFF[l] + qb0 : LOFF[l] + qb0 + P]
                    r.ap.insert(1, [qstride, nqc])
                    return r

                # token-major AV accumulators: [q, d] with a denominator
                # column, both heads -> full 128 PSUM partitions per column.
                # Slot stride padded to 128 so no slice straddles a PSUM bank.
                avb = av_ps.tile([P, 8, P], F32, tag="avb")
                ats = {}
                for g0_ in range(0, nch, kgroup):
                    gch = list(range(g0_, min(g0_ + kgroup, nch)))
                    # score chunks in pairs: one 2-bank PSUM tile, one exp
                    # instruction per pair (amortizes ScalarE per-op cost)
                    for i0 in range(0, len(gch), 2):
                        pair = gch[i0 : i0 + 2]
                        for h in (0, 1):
                            b = h * HD
                            sp = sc_ps.tile([P, 2 * qbw], F32, tag="sc")
                            for j, kc in enumerate(pair):
                                nc.tensor.matmul(
                                    sp[:, j * qbw : (j + 1) * qbw],
                                    lhsT=K[b : b + HD, LOFF[l] + kc * P : LOFF[l] + (kc + 1) * P],
                                    rhs=qrhs(b),
                                    start=True,
                                    stop=True,
                                )
                            at = at_pool.tile([P, 2 * qbw], BF16, tag="at")
                            nc.scalar.activation(
                                at[:, 0 : len(pair) * qbw],
                                sp[:, 0 : len(pair) * qbw],
                                mybir.ActivationFunctionType.Exp,
                            )
                            for j, kc in enumerate(pair):
                                ats[(kc, h)] = at[:, j * qbw : (j + 1) * qbw]
                    if filler is not None:
                        filler()
                    for kc in gch:
                        for slot in range(2 * nqc):
                            qc, h = slot // 2, slot % 2
                            c0 = 0 if h == 0 else HD + 2
                            # one start/stop per 2KB PSUM bank (4 slots):
                            # start marks the whole bank pending-zero, so
                            # the other slots' first writes self-zero.
                            nc.tensor.matmul(
                                avb[:, slot, 0 : HD + 1],
                                lhsT=ats[(kc, h)][:, qc * P : (qc + 1) * P],
                                rhs=Vt[:, CHOFF[l] + kc, c0 : c0 + HD + 1],
                                start=(kc == 0 and slot % 4 == 0),
                                stop=(
                                    kc == nch - 1
                                    and (slot % 4 == 3 or slot == 2 * nqc - 1)
                                ),
                            )
                    if filler is not None:
                        filler()

                # normalize (per-partition reciprocal of the denominator
                # column), pack both heads side by side, transpose back to
                # feature-major via the PE, copy into the destination buffer
                for qc in range(nqc):
                    sb = nrm_pool.tile([P, P], BF16, tag="sb")
                    for h in (0, 1):
                        slot = qc * 2 + h
                        r = nrm_pool.tile([P, 1], F32, tag=f"r{h}", name="rcp")
                        nc.vector.reciprocal(
                            r[:, 0:1], avb[:, slot, HD : HD + 1]
                        )
                        nc.vector.tensor_mul(
                            out=sb[:, h * HD : (h + 1) * HD],
                            in0=avb[:, slot, 0:HD],
                            in1=r[:, 0:1].to_broadcast((P, HD)),
                        )
                    tp = tr_ps.tile([P, P], BF16, tag="tr")
                    nc.tensor.transpose(tp[:], sb[:], ident[:])
                    nc.vector.tensor_copy(
                        out=a_dst[:, a_off + qc * P : a_off + (qc + 1) * P],
                        in_=tp[:],
                    )

            A123 = qk_pool.tile([P, CTOT], BF16, tag="A123")

            def attn_level_whole(l, work, cap=4):
                """Levels 1..3: write into the fused concat buffer (padded)."""
                sl = SL[l]
                co = CO[l]
                qbw = min(512, sl)
                nblk = sl // qbw
                calls = 2 * nblk * -(-CH[l] // kgroup)
                per = min(cap, -(-len(work) // calls)) if work else 0
                for qb0 in range(0, sl, qbw):
                    attn_block(
                        l, qb0, qbw, A123, co + PAD + qb0,
                        filler=lambda: drain(work, per),
                    )
                nc.vector.tensor_copy(
                    out=A123[:, co : co + PAD],
                    in_=A123[:, co + PAD : co + PAD + 1].to_broadcast((P, PAD)),
                )
                nc.vector.tensor_copy(
                    out=A123[:, co + PAD + sl : co + 2 * PAD + sl],
                    in_=A123[:, co + PAD + sl - 1 : co + PAD + sl].to_broadcast((P, PAD)),
                )

            def bounce_windows(l, dst_dram, woff):
                """One DMA materializing all 8 overlapping dest windows."""
                s0 = CO[l] + PAD - HALO[l]
                src = A123[:, s0 : s0 + WIN[l]]
                src.ap.insert(1, [BLK[l], NCORES])
                dst = dst_dram.ap().rearrange("d p w -> p d w")
                nc.sync.dma_start(dst[:, :, woff : woff + WIN[l]], src)

            def a2a(ins_t, outs_t):
                nc.gpsimd.collective_compute(
                    "AllToAll",
                    mybir.AluOpType.bypass,
                    replica_groups=rg,
                    ins=[ins_t[:]],
                    outs=[outs_t[:]],
                )

            def attn_level0_pass(work, half, ag, gout):
                """One strided half-pass of level 0: the q-set is the
                `half`-th 128-token half of every dest core's 256-block.
                Two 512-wide blocks (dests 0-3, dests 4-7), then bounce+a2a."""
                calls = 2 * NCK0 // 2 * -(-CH[0] // kgroup)
                per = min(4, -(-len(work) // calls)) if work else 0
                for b in range(2):
                    A0 = a0_pool.tile([P, QB0], BF16, tag="A0")
                    attn_block(
                        0,
                        b * 4 * BLK[0] + half * HB,
                        QB0,
                        A0,
                        0,
                        filler=lambda: drain(work, per),
                        qstride=BLK[0],
                    )
                    # 4 dests x 128 cols per block
                    nc.sync.dma_start(
                        ag.ap()[b * 4 : (b + 1) * 4].rearrange("d p w -> p d w"),
                        A0[:].rearrange("p (d w) -> p d w", d=4),
                    )
                a2a(ag, gout)

            # ---------------- epilogue ------------------------------------
            # Z_l = (gathered A_l window) @ D_l at level-l resolution, then
            # chained 2x upsample + accumulate on DVE:
            #   acc_3 = Z_3; acc_l = U(acc_{l+1}) + Z_l; out = acc_0 + beta
            def z_level(gtile, goff, l, w):
                """Matmul Z_l -> list of psum tiles (one per ft)."""
                zt = acc_pool.tile([P, FT, w], BF16, tag=f"z{l}")
                for ft in range(FT):
                    ps = ep_ps.tile([P, w], F32, tag="ep")
                    for c in range(ECH):
                        nc.tensor.matmul(
                            ps[:],
                            lhsT=wd_sb[:, l, c, ft],
                            rhs=gtile[:, c, goff : goff + w],
                            start=(c == 0),
                            stop=(c == ECH - 1),
                        )
                    nc.vector.tensor_copy(out=zt[:, ft, :], in_=ps[:])
                return zt

            def z_level_add(gtile, goff, l, w, up, extra):
                """Z_l matmuls, then out_tile = psum + up (+ extra bias)."""
                res = acc_pool.tile([P, FT, w], F32 if l == 0 else BF16, tag=f"acc{l}")
                for ft in range(FT):
                    ps = ep_ps.tile([P, w], F32, tag="ep")
                    for c in range(ECH):
                        nc.tensor.matmul(
                            ps[:],
                            lhsT=wd_sb[:, l, c, ft],
                            rhs=gtile[:, c, goff : goff + w],
                            start=(c == 0),
                            stop=(c == ECH - 1),
                        )
                    nc.vector.tensor_tensor(
                        res[:, ft, :], ps[:], up[:, ft, :], mybir.AluOpType.add
                    )
                    if extra is not None:
                        nc.vector.tensor_tensor(
                            res[:, ft, :],
                            res[:, ft, :],
                            extra[:, ft : ft + 1].to_broadcast((P, w)),
                            mybir.AluOpType.add,
                        )
                return res

            def upsample(cur, ws, w, phase_a, tag):
                """2x linear-interp upsample [P, FT, ws] -> [P, FT, w] (DVE)."""
                p25 = acc_pool.tile([P, FT, ws], BF16, tag=f"p25{tag}")
                p75 = acc_pool.tile([P, FT, ws], BF16, tag=f"p75{tag}")
                nc.vector.tensor_scalar_mul(p25[:], cur[:], 0.25)
                nc.vector.tensor_scalar_mul(p75[:], cur[:], 0.75)
                up = acc_pool.tile([P, FT, w], BF16, tag=f"up{tag}")
                hw = (w + 1) // 2
                hw2 = w // 2
                if phase_a:
                    nc.vector.tensor_add(
                        up[:, :, 0::2], p25[:, :, 0:hw], p75[:, :, 1 : hw + 1]
                    )
                    nc.vector.tensor_add(
                        up[:, :, 1::2], p75[:, :, 1 : hw2 + 1], p25[:, :, 2 : hw2 + 2]
                    )
                else:
                    nc.vector.tensor_add(
                        up[:, :, 0::2], p75[:, :, 1 : hw + 1], p25[:, :, 2 : hw + 2]
                    )
                    nc.vector.tensor_add(
                        up[:, :, 1::2], p25[:, :, 1 : hw2 + 1], p75[:, :, 2 : hw2 + 2]
                    )
                return up

            # ---------------- schedule ------------------------------------
            # level order 3, 2, 1, 0: the fused 3+2 exchange and the level-1
            # exchange go out during attention; level 0 runs last as two
            # strided half-passes so its first half-exchange hides under the
            # second half-pass and only the second (small) one is exposed,
            # followed by just half of Z_0 + final adds.
            drain(qkv_chunks(3))
            w2 = qkv_chunks(2)
            w1 = qkv_chunks(1)
            w0 = qkv_chunks(0)
            attn_level_whole(3, w2, cap=5)
            drain(w2)
            attn_level_whole(2, w1, cap=7)
            bounce_windows(3, agin32, 0)
            bounce_windows(2, agin32, WIN[3])
            a2a(agin32, g32)
            Gs32 = g_pool.tile([P, ECH, W32], BF16, tag="gs32")
            nc.sync.dma_start(Gs32[:], g32.ap().rearrange("b p t -> p b t"))
            drain(w1)
            attn_level_whole(1, w0, cap=6)
            bounce_windows(1, agin1, 0)
            a2a(agin1, g1)
            Gs1 = g_pool.tile([P, ECH, WIN[1]], BF16, tag="gs1")
            nc.sync.dma_start(Gs1[:], g1.ap().rearrange("b p t -> p b t"))
            drain(w0)
            attn_level0_pass([], 0, agin0a, g0a)
            attn_level0_pass([], 1, agin0b, g0b)

            stackA.close()
            acc_pool = ctx.enter_context(tc.tile_pool(name="acc", bufs=1))
            ep_ps = ctx.enter_context(tc.tile_pool(name="ep_ps", bufs=2, space="PSUM"))

            # chain part that needs only g32/g1 (both landed during attention)
            acc = z_level(Gs32, 0, 3, WIN[3])
            up = upsample(acc, WIN[3], WIN[2], cfg["PHASE_A"][2], "a")
            acc = z_level_add(Gs32, WIN[3], 2, WIN[2], up, None)
            upb = upsample(acc, WIN[2], WIN[1], cfg["PHASE_A"][1], "b")
            acc = z_level_add(Gs1, 0, 1, WIN[1], upb, None)
            upc = upsample(acc, WIN[1], WIN[0], cfg["PHASE_A"][0], "c")

            # level-0 halves: matmul + (beta, upc) adds + streamed out DMA
            out_r = out_p.ap().rearrange("(c p) t -> p c t", p=P)
            for half, gsrc in ((0, g0a), (1, g0b)):
                gs = g_pool.tile(
                    [P, ECH, HB], BF16, tag=f"gs0{half}", name=f"gs0{half}"
                )
                nc.sync.dma_start(gs[:], gsrc.ap().rearrange("b p t -> p b t"))
                for ft in range(FT):
                    ps = ep_ps.tile([P, HB], F32, tag="ep")
                    for c in range(ECH):
                        nc.tensor.matmul(
                            ps[:],
                            lhsT=wd_sb[:, 0, c, ft],
                            rhs=gs[:, c, :],
                            start=(c == 0),
                            stop=(c == ECH - 1),
                        )
                    o = acc_pool.tile([P, HB], F32, tag=f"o{half}_{ft}", name="o_t")
                    nc.vector.tensor_tensor(
                        o[:],
                        ps[:],
                        beta_sb[:, ft : ft + 1].to_broadcast((P, HB)),
                        mybir.AluOpType.add,
                    )
                    nc.vector.tensor_tensor(
                        o[:],
                        o[:],
                        upc[:, ft, half * HB : (half + 1) * HB],
                        mybir.AluOpType.add,
                    )
                    nc.sync.dma_start(
                        out_r[:, ft, half * HB : (half + 1) * HB], o[:]
                    )

    nc.compile()
    return nc


# ---------------------------------------------------------------------------
# host-side input preparation / sharding
# ---------------------------------------------------------------------------

def make_in_maps(cfg, query, in_proj_w, in_proj_b, out_w, out_b, up_w, up_b):
    S, E, HD, F, ECH = cfg["S"], cfg["E"], cfg["HD"], cfg["F"], cfg["ECH"]
    FT = ECH
    f32 = np.float32
    f64 = np.float64

    query = np.asarray(query, f32)
    in_proj_w = np.asarray(in_proj_w, f32)
    in_proj_b = np.asarray(in_proj_b, f32)
    out_w = np.asarray(out_w, f32)
    out_b = np.asarray(out_b, f32)
    up_w = np.asarray(up_w, f32)
    up_b = np.asarray(up_b, f32)

    qT = np.ascontiguousarray(query[0].T.astype(BF16_NP))  # [E, S]

    # folded epilogue: D_l = W_out[l]^T @ up_w[l-1]^T @ ... @ up_w[0]^T
    # beta: beta_3 = b3; beta_l = beta_{l+1} @ up_w[l]^T + up_b[l] + b_l
    D = []
    for l in range(LEVELS):
        M = out_w[l].T.astype(f64)
        for j in range(l - 1, -1, -1):
            M = M @ up_w[j].T.astype(f64)
        D.append(M.astype(f32))
    Dm = np.stack(D, axis=0)  # [L, E(in), E(out)] -- already W^T layout
    beta = out_b[3].astype(f64)
    for l in range(LEVELS - 2, -1, -1):
        beta = beta @ up_w[l].T.astype(f64) + up_b[l] + out_b[l]
    beta = beta.astype(f32)

    # pack [L, e_in, e_out] -> [L, e_in%128, e_in//128, e_out//128, e_out%128]
    t = Dm.reshape(LEVELS, ECH, P, FT, P)          # [L, ec, ep, ft, fp]
    t = t.transpose(0, 2, 1, 3, 4)                 # [L, ep, ec, ft, fp]
    wd = np.ascontiguousarray(t.astype(BF16_NP))
    beta_pk = np.ascontiguousarray(beta.reshape(FT, P).T.astype(f32))  # [P, FT]

    scale = 1.0 / np.sqrt(HD).astype(f32)
    in_maps = []
    for c in range(NCORES):
        r0 = c * F
        sl_q = in_proj_w[:, r0 : r0 + F, :] * scale          # [L, F, E]
        sl_k = in_proj_w[:, E + r0 : E + r0 + F, :]
        sl_v = in_proj_w[:, 2 * E + r0 : 2 * E + r0 + F, :]
        w3 = np.stack([sl_q, sl_k, sl_v], axis=1)            # [L, 3, F, E]
        w3 = w3.transpose(0, 3, 1, 2)                        # [L, E(e), 3, F]
        w3 = w3.reshape(LEVELS, ECH, P, 3, F).transpose(0, 2, 3, 1, 4)
        w3 = np.ascontiguousarray(w3.astype(BF16_NP))        # [L, p, 3, ch, F]

        b_q = in_proj_b[:, r0 : r0 + F] * scale
        b_k = in_proj_b[:, E + r0 : E + r0 + F]
        b_v = in_proj_b[:, 2 * E + r0 : 2 * E + r0 + F]
        b3 = np.stack([b_q, b_k, b_v], axis=1)               # [L, 3, F]
        b3 = np.zeros((P, LEVELS, 3), f32) + b3.transpose(2, 0, 1)

        in_maps.append(
            {
                "qT": qT,
                "win": w3,
                "bin": np.ascontiguousarray(b3),
                "wd": wd,
                "beta": beta_pk,
            }
        )
    return in_maps


def assemble_output(cfg, results):
    S, E = cfg["S"], cfg["E"]
    blk = cfg["BLK"][0]
    out = np.empty((1, S, E), np.float32)
    for c in range(NCORES):
        out[0, c * blk : (c + 1) * blk, :] = results[c]["out"].T
    return out


_CACHE = {}


def _get_nc(cfg_key=(2048, 1024, 16)):
    if cfg_key not in _CACHE:
        cfg = _cfg(*cfg_key)
        _CACHE[cfg_key] = (cfg, build(cfg))
    return _CACHE[cfg_key]


def kernel(query, in_proj_w, in_proj_b, out_w, out_b, up_w, up_b):
    from concourse.bass_utils import run_bass_kernel_spmd

    cfg, nc = _get_nc()
    in_maps = make_in_maps(cfg, query, in_proj_w, in_proj_b, out_w, out_b, up_w, up_b)
    res = run_bass_kernel_spmd(nc, in_maps, core_ids=list(range(NCORES)))
    return assemble_output(cfg, res.results)
